# revision 3
# baseline (speedup 1.0000x reference)
"""Trainium2 Bass kernel for cosine-similarity ("sparse") attention.

Reference computation (B=2, C=512, N=2048, H=16, D=64, SCALE=8):
    qkv = Wqkv @ x                          # 1x1 conv
    q,k,v -> [B,H,D,N]
    q = l2norm(q, over D) * q_scale ; k = l2norm(k, over D) * k_scale
    sim = (q^T k) * 8 ; attn = softmax(sim, over keys)
    out = Wout @ (attn @ v) + bout

Sharding: 32 (batch, head) pairs across 8 cores -> each core owns one batch
(b = core//4) and 4 heads (h0 = 4*(core%4)).  Each core projects q/k/v for
its heads, runs attention, and computes a partial output projection
Wout[:, its-heads] @ y + bout/4.  Host sums the 4 partials per batch.

Device-side schedule (per core):
  - fp16 for every 16-bit tensor (x, W, qn/kn, vext, at, y, wo): fp16's 10
    mantissa bits beat bf16 ~8x on this problem, which buys the error
    budget for the fast-exp trick below.
  - The [128,1024] softmax exp (2 heads x 512 queries per j-step) is
    split per head: ACT does head A with the exact Exp table; DVE does
    head B with a Schraudolph bit-trick exp: i16 = round(sim*1024*log2e
    + (15*1024 - 44)) via one tensor_scalar (fp32 PSUM -> int16), the
    int16 tile being the fp16 `at` tile bitcast.  ~+-4% on those columns,
    but softmax self-normalisation cancels the common mode; end-to-end
    rel err ~8e-3 (gate 2e-2).
  - sim is two per-head [128,512] PSUM tiles (bufs=4) so the engines
    never share a tile and block handoffs don't stall on bank rotation.
  - Softmax denominator: ones column in vext -> row D of the av
    accumulator.  Chain: ACT Copy evacuates rows [0:64] and row 64 (ACT
    handles partition-offset PSUM reads; custom-DVE ops and gpsimd
    partition_broadcast do NOT -- verified on HW), DVE recip, gpsimd
    broadcast, gpsimd multiply into fp16 y (gpsimd has no PSUM port).
  - out-proj bias rides the PSUM accumulation as a third matmul
    (bias_row x ones_row), so the evacuation is a plain ACT Copy (Copy
    shares the exp table set -- no ACT table thrash).
  - The av pair runs one j-step behind the sim pair ACROSS block
    boundaries; out-proj column chunks are staggered one per 2 j-steps;
    the last block flushes immediately to shorten the tail.
  - Softmax max-subtraction is skipped: sim = 8*cosine is in [-8, 8]
    (which also keeps the Schraudolph i16 strictly positive).
  - l2norm: ones-indicator matmul for sumsq, fused PSUM->SBUF Sqrt on
    ACT, DVE reciprocal, inverse-norm broadcast via a DMA roundtrip.
  - Startup: x rides one DMA per 128-partition chunk; wq first so the
    first projection starts as early as possible.
"""

import os
import sys

import numpy as np

sys.path.insert(0, "/opt/trn_rl_repo")

import concourse.bass as bass  # noqa: E402
import concourse.mybir as mybir  # noqa: E402
from concourse import bacc, tile  # noqa: E402
from concourse.bass_utils import run_bass_kernel_spmd  # noqa: E402

F32 = mybir.dt.float32
F32R = mybir.dt.float32r
F16 = mybir.dt.float16
I16 = mybir.dt.int16
AF = mybir.ActivationFunctionType
OP = mybir.AluOpType

B, C, N = 2, 512, 2048
HEADS, D = 16, 64
SCALE = 8.0
NCORES = 8
HPC = 4  # heads per core

# Schraudolph fast-exp constants (fp16 bit layout):
# i16 = round(sim * S_SCH + B_SCH); bits reinterpreted as fp16 ~= e^sim.
S_SCH = 1024.0 / float(np.log(2.0))
B_SCH = 15.0 * 1024.0 - 44.0

_CACHED_NC = None
LAST_RESULTS = None
EXTRA_RUN_KWARGS = {}


def build_nc():
    nc = bacc.Bacc(None, target_bir_lowering=False)

    x_d = nc.declare_dram_parameter("x", [C, N], F16, isOutput=False)
    wqT_d = nc.declare_dram_parameter("wqT", [C, HPC * D], F16, isOutput=False)
    wkT_d = nc.declare_dram_parameter("wkT", [C, HPC * D], F16, isOutput=False)
    wvT_d = nc.declare_dram_parameter("wvT", [C, HPC * D], F16, isOutput=False)
    woT_d = nc.declare_dram_parameter("woT", [HPC * D, C], F16, isOutput=False)
    qsks8_d = nc.declare_dram_parameter("qsks8", [128, 1], F32, isOutput=False)
    onesw_d = nc.declare_dram_parameter("onesw", [128, 33], F32R, isOutput=False)
    biasr_d = nc.declare_dram_parameter("biasr", [1, C], F16, isOutput=False)
    out_d = nc.declare_dram_parameter("out", [C, N], F32, isOutput=True)

    NQT = N // 512  # 4 query chunks of 512
    NJ = N // 128  # 16 key chunks of 128
    NCT = C // 128  # 4 channel chunks of 128

    with tile.TileContext(nc) as tc:
        with (
            tc.tile_pool(name="const", bufs=1) as const,
            tc.tile_pool(name="persist", bufs=1) as persist,
            tc.tile_pool(name="dramp", bufs=1, space="DRAM") as dramp,
        ):
            qsks8_sb = const.tile([128, 1], F32, name="qsks8", tag="qsks8")
            nc.sync.dma_start(qsks8_sb[:], qsks8_d[:])
            biasr_sb = const.tile([1, C], F16, name="biasr", tag="biasr")
            nc.sync.dma_start(biasr_sb[:], biasr_d[:])
            ones512 = const.tile([1, 512], F16, name="ones512", tag="ones512")
            nc.gpsimd.memset(ones512[:], 1.0)
            # indicator weights: col 0 sums partitions 0-63 (head A), col 32
            # sums partitions 64-127 (head B); middle cols write zeros so the
            # [33, 512] sumsq psum rows land 32-aligned.
            ones_w = const.tile([128, 33], F32R, name="ones_w", tag="ones_w")
            nc.sync.dma_start(ones_w[:], onesw_d[:])
            wo_sb = [
                const.tile([128, C], F16, name=f"wo{m}", tag=f"wo{m}")
                for m in range(2)
            ]

            # persistent tensors
            qn = [persist.tile([128, N], F16, name=f"qn{m}", tag=f"qn{m}") for m in range(2)]
            kn = [persist.tile([128, N], F16, name=f"kn{m}", tag=f"kn{m}") for m in range(2)]
            y = [
                [
                    persist.tile([128, 512], F16, name=f"y{m}_{qt}", tag=f"y{m}_{qt}")
                    for qt in range(4)
                ]
                for m in range(2)
            ]
            vext = persist.tile([128, NJ, HPC, D + 1], F16, name="vext", tag="vext")
            inv_dram = dramp.tile([8, N], F32, name="inv_dram", tag="inv_dram")
            # softmax-denominator ones column: engine memset
            nc.gpsimd.memset(vext[:, :, :, D : D + 1], 1.0)

            # ---------------- phase 1: projections + norms ----------------
            with (
                tc.tile_pool(name="xw", bufs=1) as xw,
                tc.tile_pool(name="raw", bufs=1) as rawp,
                tc.tile_pool(name="sq", bufs=3) as sqp,
                tc.tile_pool(name="bb", bufs=4) as bbp,
                tc.tile_pool(name="prps", bufs=4, space="PSUM") as prps,
                tc.tile_pool(name="ssps", bufs=2, space="PSUM") as ssps,
            ):
                srt_tm = [
                    [
                        rawp.tile([33, N], F32, name=f"srt{t}{m}", tag=f"srt{t}{m}")
                        for m in range(2)
                    ]
                    for t in range(2)
                ]
                inv_tm = [
                    [
                        rawp.tile([33, N], F32, name=f"inv{t}{m}", tag=f"inv{t}{m}")
                        for m in range(2)
                    ]
                    for t in range(2)
                ]
                # DMA order tuned for earliest first matmul: wq, x[c0,c1],
                # wk, x[c2,c3], wv, wo; one DMA per x chunk row.
                wq_all = xw.tile([128, NCT, HPC * D], F16, name="wq_all", tag="wq_all")
                nc.scalar.dma_start(
                    wq_all[:], wqT_d[:].rearrange("(c p) d -> p c d", p=128)
                )
                wq_sb = [wq_all[:, c, :] for c in range(NCT)]
                dma_engs = [nc.sync, nc.scalar, nc.gpsimd, nc.sync]
                x_all = []
                for c in range(NCT):
                    t = xw.tile([128, N], F16, name=f"x{c}", tag=f"x{c}")
                    dma_engs[c].dma_start(t[:], x_d[c * 128 : (c + 1) * 128, :])
                    x_all.append(t)
                x_sb = [
                    [x_all[c][:, nt * 512 : (nt + 1) * 512] for nt in range(NQT)]
                    for c in range(NCT)
                ]
                wk_all = xw.tile([128, NCT, HPC * D], F16, name="wk_all", tag="wk_all")
                nc.gpsimd.dma_start(
                    wk_all[:], wkT_d[:].rearrange("(c p) d -> p c d", p=128)
                )
                wk_sb = [wk_all[:, c, :] for c in range(NCT)]
                wv_all = xw.tile([128, NCT, HPC * D], F16, name="wv_all", tag="wv_all")
                nc.scalar.dma_start(
                    wv_all[:], wvT_d[:].rearrange("(c p) d -> p c d", p=128)
                )
                wv_sb = [wv_all[:, c, :] for c in range(NCT)]
                for m in range(2):
                    nc.gpsimd.dma_start(
                        wo_sb[m][:], woT_d[m * 128 : (m + 1) * 128, :]
                    )

                # sumsq matmuls are emitted one proj-chunk late so the PE
                # never waits on the ACT square of the current chunk
                pend_ss = []

                def emit_ss(limit):
                    while len(pend_ss) > limit:
                        sq_t, ti_, m_, nt_ = pend_ss.pop(0)
                        ss = ssps.tile([33, 512], F32, name="ss", tag="ss")
                        nc.tensor.matmul(
                            ss[:], lhsT=(ones_w[:]), rhs=(sq_t[:]), start=True, stop=True
                        )
                        nc.scalar.activation(
                            srt_tm[ti_][m_][:, nt_ * 512 : (nt_ + 1) * 512],
                            ss[:],
                            AF.Sqrt,
                        )

                def proj_chunk(m, w_sb, raws, ti, nt):
                    ps = prps.tile([128, 512], F32, name="pr", tag="pr")
                    for c in range(NCT):
                        nc.tensor.matmul(
                            ps[:],
                            lhsT=(w_sb[c][:, m * 128 : (m + 1) * 128]),
                            rhs=(x_sb[c][nt]),
                            start=(c == 0),
                            stop=(c == NCT - 1),
                        )
                    emit_ss(1)
                    nc.vector.tensor_copy(
                        raws[m][:, nt * 512 : (nt + 1) * 512], ps[:]
                    )
                    sq = sqp.tile([128, 512], F32R, name="sq", tag="sq")
                    nc.scalar.activation(sq[:], ps[:], AF.Square)
                    pend_ss.append((sq, ti, m, nt))

                def norm_head(m):
                    # reciprocals + the inverse-norm row DMA roundtrip with
                    # the 64-partition broadcast.
                    bts = []
                    for ti in range(2):
                        nc.vector.reciprocal_approx_fast(
                            inv_tm[ti][m][:], srt_tm[ti][m][:]
                        )
                        nc.sync.dma_start(
                            inv_dram[4 * ti + 2 * m : 4 * ti + 2 * m + 2, :],
                            inv_tm[ti][m][0:33:32, :],
                        )
                    bt_engs = [nc.sync, nc.gpsimd]
                    for ti in range(2):
                        rowA = 4 * ti + 2 * m
                        bt = bbp.tile([128, N], F32, name="bt", tag="bt")
                        eng = bt_engs[ti]
                        eng.dma_start(
                            bt[0:64, :].unsqueeze(1),
                            inv_dram[rowA : rowA + 1, :].partition_broadcast(64),
                        )
                        eng.dma_start(
                            bt[64:128, :].unsqueeze(1),
                            inv_dram[rowA + 1 : rowA + 2, :].partition_broadcast(64),
                        )
                        bts.append(bt)
                    return bts

                def norm_tail(m, bts):
                    nc.vector.scalar_tensor_tensor(
                        qn[m][:], qn[m][:], qsks8_sb[:], bts[0][:],
                        OP.mult, OP.mult,
                    )
                    nc.gpsimd.tensor_tensor(
                        kn[m][:], kn[m][:], bts[1][:], OP.mult
                    )

                for nt in range(NQT):
                    proj_chunk(0, wq_sb, qn, 0, nt)
                    proj_chunk(0, wk_sb, kn, 1, nt)
                emit_ss(0)
                bts0 = norm_head(0)
                for nt in range(NQT):
                    proj_chunk(1, wq_sb, qn, 0, nt)
                    proj_chunk(1, wk_sb, kn, 1, nt)
                    if nt == 2:
                        norm_tail(0, bts0)
                emit_ss(0)
                bts1 = norm_head(1)

                # v projection; vext copies alternate ACT/DVE so neither
                # engine is the phase-1 straggler
                for nm_ in range(NJ):
                    psv = prps.tile([128, HPC * D], F32, name="prv", tag="pr")
                    for c in range(NCT):
                        nc.tensor.matmul(
                            psv[:],
                            lhsT=(
                                x_all[c][
                                    :, (nm_ * 128) : (nm_ * 128 + 128)
                                ]
                            ),
                            rhs=(wv_sb[c][:]),
                            start=(c == 0),
                            stop=(c == NCT - 1),
                        )
                    if nm_ % 2 == 0:
                        nc.scalar.activation(
                            vext[:, nm_, :, 0:D],
                            psv[:].rearrange("p (h d) -> p h d", h=HPC),
                            AF.Copy,
                        )
                    else:
                        nc.vector.tensor_copy(
                            vext[:, nm_, :, 0:D],
                            psv[:].rearrange("p (h d) -> p h d", h=HPC),
                        )
                norm_tail(1, bts1)

            # ---------------- phase 2: attention + fused out-proj ----------
            with (
                tc.tile_pool(name="simps", bufs=4, space="PSUM") as simps,
                tc.tile_pool(name="ops", bufs=3, space="PSUM") as ops,
                tc.tile_pool(name="ppps", bufs=1, space="PSUM") as ppps,
                tc.tile_pool(name="at", bufs=4) as atp,
                tc.tile_pool(name="nrm", bufs=4) as nrm,
                tc.tile_pool(name="fin", bufs=4) as finp,
            ):
                out_engs = [nc.sync, nc.gpsimd, nc.sync, nc.gpsimd]

                def out_proj_halves(qt, ct):
                    qs_ = slice(qt * 512, (qt + 1) * 512)
                    box = {}

                    def start_half():
                        pp = ppps.tile([128, 512], F32, name="pp", tag="pp")
                        box["pp"] = pp
                        nc.tensor.matmul(
                            pp[:],
                            lhsT=(wo_sb[0][:, ct * 128 : (ct + 1) * 128]),
                            rhs=(y[0][qt][:]),
                            start=True,
                            stop=False,
                        )
                        # bias folded into the accumulation: bias_row x ones
                        nc.tensor.matmul(
                            pp[:],
                            lhsT=(biasr_sb[:, ct * 128 : (ct + 1) * 128]),
                            rhs=(ones512[:]),
                            start=False,
                            stop=False,
                        )

                    def finish_half():
                        pp = box["pp"]
                        nc.tensor.matmul(
                            pp[:],
                            lhsT=(wo_sb[1][:, ct * 128 : (ct + 1) * 128]),
                            rhs=(y[1][qt][:]),
                            start=False,
                            stop=True,
                        )
                        ot = finp.tile([128, 512], F32, name="ot", tag="ot")
                        nc.scalar.activation(ot[:], pp[:], AF.Copy)
                        out_engs[ct].dma_start(
                            out_d[ct * 128 : (ct + 1) * 128, qs_], ot[:]
                        )

                    return start_half, finish_half

                def av_pair(at_t, j_, oA, oB, hA, hB):
                    nc.tensor.matmul(
                        oA[:],
                        lhsT=(vext[:, j_, hA, :]),
                        rhs=(at_t[:, 0:512]),
                        start=(j_ == 0),
                        stop=(j_ == NJ - 1),
                    )
                    nc.tensor.matmul(
                        oB[:],
                        lhsT=(vext[:, j_, hB, :]),
                        rhs=(at_t[:, 512:1024]),
                        start=(j_ == 0),
                        stop=(j_ == NJ - 1),
                    )

                def normalize(oA, oB, m, qt):
                    # ACT evacuates the accumulator (rows 0:64) and the
                    # denominator row separately (offset-64 PSUM reads are
                    # only correct on ACT); DVE recips; gpsimd broadcasts
                    # and multiplies into fp16 y (gpsimd cannot touch PSUM).
                    for o_ps, base in ((oA, 0), (oB, 64)):
                        oc = nrm.tile([64, 512], F32, name="oc", tag="oc")
                        nc.scalar.activation(oc[:], o_ps[0:D, :], AF.Copy)
                        rsb = nrm.tile([1, 512], F32, name="rsb", tag="rsb")
                        nc.scalar.activation(rsb[:], o_ps[D : D + 1, :], AF.Copy)
                        inv = nrm.tile([1, 512], F32, name="inv", tag="inv")
                        nc.vector.reciprocal_approx_fast(inv[:], rsb[:])
                        br = nrm.tile([64, 512], F32, name="br", tag="br")
                        nc.gpsimd.partition_broadcast(br[:], inv[:], channels=64)
                        nc.gpsimd.tensor_tensor(
                            y[m][qt][base : base + 64, :],
                            oc[:],
                            br[:],
                            OP.mult,
                        )

                pend_av = [None]
                pend_pp = []
                astep = [0]

                def flush_pend(last=False):
                    if pend_av[0] is None:
                        return
                    at_t, j_, oA, oB, m_, qt_ = pend_av[0]
                    pend_av[0] = None
                    av_pair(at_t, j_, oA, oB, 2 * m_, 2 * m_ + 1)
                    if j_ == NJ - 1:
                        normalize(oA, oB, m_, qt_)
                        if m_ == 1:
                            for ct in range(NCT):
                                for fn in out_proj_halves(qt_, ct):
                                    if last:
                                        fn()
                                    else:
                                        pend_pp.append((astep[0] + 5, fn))

                for qt in range(NQT):
                    for m in range(2):
                        qs_ = slice(qt * 512, (qt + 1) * 512)
                        oA = ops.tile([D + 1, 512], F32, name="oA", tag="o")
                        oB = ops.tile([D + 1, 512], F32, name="oB", tag="o")
                        for j in range(NJ):
                            js = slice(j * 128, (j + 1) * 128)
                            simA = simps.tile([128, 512], F32, name="simA", tag="sim")
                            simB = simps.tile([128, 512], F32, name="simB", tag="sim")
                            nc.tensor.matmul(
                                simA[:],
                                lhsT=(kn[m][0:64, js]),
                                rhs=(qn[m][0:64, qs_]),
                                start=True,
                                stop=True,
                            )
                            nc.tensor.matmul(
                                simB[:],
                                lhsT=(kn[m][64:128, js]),
                                rhs=(qn[m][64:128, qs_]),
                                start=True,
                                stop=True,
                            )
                            flush_pend()
                            if pend_pp and j % 2 == 1 and astep[0] >= pend_pp[0][0]:
                                pend_pp.pop(0)[1]()
                            at = atp.tile([128, 1024], F16, name="at", tag="at")
                            # exact exp on ACT for head A
                            nc.scalar.activation(at[:, 0:512], simA[:], AF.Exp)
                            # Schraudolph fast-exp on DVE for head B
                            nc.vector.tensor_scalar(
                                at[:, 512:1024].bitcast(I16),
                                simB[:],
                                S_SCH,
                                B_SCH,
                                op0=OP.mult,
                                op1=OP.add,
                            )
                            pend_av[0] = (at, j, oA, oB, m, qt)
                            astep[0] += 1
                flush_pend(last=True)
                while pend_pp:
                    pend_pp.pop(0)[1]()

    nc.finalize()
    return nc


def kernel(x, Wqkv, q_scale, k_scale, Wout, bout):
    global _CACHED_NC, LAST_RESULTS
    x = np.asarray(x, dtype=np.float32)
    Wqkv = np.asarray(Wqkv, dtype=np.float32)
    q_scale = np.asarray(q_scale, dtype=np.float32)
    k_scale = np.asarray(k_scale, dtype=np.float32)
    Wout = np.asarray(Wout, dtype=np.float32)
    bout = np.asarray(bout, dtype=np.float32)

    if _CACHED_NC is None:
        _CACHED_NC = build_nc()
    nc = _CACHED_NC

    H_DIM = HEADS * D
    qsks8 = np.tile((SCALE * q_scale * k_scale).astype(np.float32), 2)[:, None]
    qsks8 = np.ascontiguousarray(qsks8)
    biasr = np.ascontiguousarray((bout / 4.0).astype(np.float16)[None, :])
    onesw = np.zeros((128, 33), dtype=np.float32)
    onesw[0:64, 0] = 1.0
    onesw[64:128, 32] = 1.0

    in_maps = []
    for core in range(NCORES):
        b = core // 4
        h0 = HPC * (core % 4)
        rs = slice(h0 * D, h0 * D + HPC * D)
        wq = Wqkv[0:H_DIM][rs]
        wk = Wqkv[H_DIM : 2 * H_DIM][rs]
        wv = Wqkv[2 * H_DIM : 3 * H_DIM][rs]
        in_maps.append(
            {
                "x": np.ascontiguousarray(x[b]).astype(np.float16),
                "wqT": np.ascontiguousarray(wq.T).astype(np.float16),
                "wkT": np.ascontiguousarray(wk.T).astype(np.float16),
                "wvT": np.ascontiguousarray(wv.T).astype(np.float16),
                "woT": np.ascontiguousarray(Wout[:, rs].T).astype(np.float16),
                "qsks8": qsks8,
                "onesw": onesw,
                "biasr": biasr,
            }
        )

    res = run_bass_kernel_spmd(
        nc,
        in_maps,
        core_ids=list(range(NCORES)),
        trace=bool(os.environ.get("BASS_TRACE")),
        **EXTRA_RUN_KWARGS,
    )
    LAST_RESULTS = res

    outs = [np.asarray(res.results[i]["out"], dtype=np.float32) for i in range(NCORES)]
    full = np.empty((B, C, N), dtype=np.float32)
    full[0] = outs[0] + outs[1] + outs[2] + outs[3]
    full[1] = outs[4] + outs[5] + outs[6] + outs[7]
    return full


# revision 10
# speedup vs baseline: 1.0106x; 1.0106x over previous
"""Trainium2 Bass kernel for cosine-similarity ("sparse") attention.

Reference computation (B=2, C=512, N=2048, H=16, D=64, SCALE=8):
    qkv = Wqkv @ x                          # 1x1 conv
    q,k,v -> [B,H,D,N]
    q = l2norm(q, over D) * q_scale ; k = l2norm(k, over D) * k_scale
    sim = (q^T k) * 8 ; attn = softmax(sim, over keys)
    out = Wout @ (attn @ v) + bout

Sharding: 32 (batch, head) pairs across 8 cores -> each core owns one batch
(b = core//4) and 4 heads (h0 = 4*(core%4)).  Each core projects q/k/v for
its heads, runs attention, and computes a partial output projection
Wout[:, its-heads] @ y + bout/4.  Host sums the 4 partials per batch.

Device-side schedule (per core):
  - fp16 for every 16-bit tensor (x, W, qn/kn, vext, at, y, wo): fp16's 10
    mantissa bits beat bf16 ~8x on this problem, which buys the error
    budget for the fast-exp trick below.
  - The [128,1024] softmax exp (2 heads x 512 queries per j-step) is
    split per head: ACT does head A with the exact Exp table; DVE does
    head B with a Schraudolph bit-trick exp: i16 = round(sim*1024*log2e
    + (15*1024 - 44)) via one tensor_scalar (fp32 PSUM -> int16), the
    int16 tile being the fp16 `at` tile bitcast.  ~+-4% on those columns,
    but softmax self-normalisation cancels the common mode; end-to-end
    rel err ~8e-3 (gate 2e-2).
  - sim is two per-head [128,512] PSUM tiles (bufs=4) so the engines
    never share a tile and block handoffs don't stall on bank rotation.
  - Softmax denominator: ones column in vext -> row D of the av
    accumulator.  Chain: ACT Copy evacuates rows [0:64] and row 64 (ACT
    handles partition-offset PSUM reads; custom-DVE ops and gpsimd
    partition_broadcast do NOT -- verified on HW), DVE recip, gpsimd
    broadcast, gpsimd multiply into fp16 y (gpsimd has no PSUM port).
  - out-proj bias rides the PSUM accumulation as a third matmul
    (bias_row x ones_row), so the evacuation is a plain ACT Copy (Copy
    shares the exp table set -- no ACT table thrash).
  - The av pair runs one j-step behind the sim pair ACROSS block
    boundaries; out-proj column chunks are staggered one per 2 j-steps;
    the last block flushes immediately to shorten the tail.
  - Softmax max-subtraction is skipped: sim = 8*cosine is in [-8, 8]
    (which also keeps the Schraudolph i16 strictly positive).
  - l2norm: ones-indicator matmul for sumsq, fused PSUM->SBUF Sqrt on
    ACT, DVE reciprocal, inverse-norm broadcast via a DMA roundtrip.
  - Startup: x rides one DMA per 128-partition chunk; wq first so the
    first projection starts as early as possible.
"""

import os
import sys

import numpy as np

sys.path.insert(0, "/opt/trn_rl_repo")

import concourse.bass as bass  # noqa: E402
import concourse.mybir as mybir  # noqa: E402
from concourse import bacc, tile  # noqa: E402
from concourse.bass_utils import run_bass_kernel_spmd  # noqa: E402

F32 = mybir.dt.float32
F32R = mybir.dt.float32r
F16 = mybir.dt.float16
I16 = mybir.dt.int16
AF = mybir.ActivationFunctionType
OP = mybir.AluOpType

B, C, N = 2, 512, 2048
HEADS, D = 16, 64
SCALE = 8.0
NCORES = 8
HPC = 4  # heads per core

# Schraudolph fast-exp constants (fp16 bit layout):
# i16 = round(sim * S_SCH + B_SCH); bits reinterpreted as fp16 ~= e^sim.
S_SCH = 1024.0 / float(np.log(2.0))
B_SCH = 15.0 * 1024.0 - 44.0

_CACHED_NC = None
LAST_RESULTS = None
EXTRA_RUN_KWARGS = {}


def build_nc():
    nc = bacc.Bacc(None, target_bir_lowering=False)

    x_d = nc.declare_dram_parameter("x", [C, N], F16, isOutput=False)
    wqT_d = nc.declare_dram_parameter("wqT", [C, HPC * D], F16, isOutput=False)
    wkT_d = nc.declare_dram_parameter("wkT", [C, HPC * D], F16, isOutput=False)
    wvT_d = nc.declare_dram_parameter("wvT", [C, HPC * D], F16, isOutput=False)
    woT_d = nc.declare_dram_parameter("woT", [HPC * D, C], F16, isOutput=False)
    qsks8_d = nc.declare_dram_parameter("qsks8", [128, 1], F32, isOutput=False)
    onesw_d = nc.declare_dram_parameter("onesw", [128, 33], F32R, isOutput=False)
    biasr_d = nc.declare_dram_parameter("biasr", [1, C], F16, isOutput=False)
    out_d = nc.declare_dram_parameter("out", [C, N], F32, isOutput=True)

    NQT = N // 512  # 4 query chunks of 512
    NJ = N // 128  # 16 key chunks of 128
    NCT = C // 128  # 4 channel chunks of 128

    with tile.TileContext(nc) as tc:
        with (
            tc.tile_pool(name="const", bufs=1) as const,
            tc.tile_pool(name="persist", bufs=1) as persist,
            tc.tile_pool(name="dramp", bufs=1, space="DRAM") as dramp,
        ):
            qsks8_sb = const.tile([128, 1], F32, name="qsks8", tag="qsks8")
            nc.sync.dma_start(qsks8_sb[:], qsks8_d[:])
            biasr_sb = const.tile([1, C], F16, name="biasr", tag="biasr")
            nc.sync.dma_start(biasr_sb[:], biasr_d[:])
            ones512 = const.tile([1, 512], F16, name="ones512", tag="ones512")
            nc.gpsimd.memset(ones512[:], 1.0)
            # indicator weights: col 0 sums partitions 0-63 (head A), col 32
            # sums partitions 64-127 (head B); middle cols write zeros so the
            # [33, 512] sumsq psum rows land 32-aligned.
            ones_w = const.tile([128, 33], F32R, name="ones_w", tag="ones_w")
            nc.sync.dma_start(ones_w[:], onesw_d[:])
            wo_sb = [
                const.tile([128, C], F16, name=f"wo{m}", tag=f"wo{m}")
                for m in range(2)
            ]

            # persistent tensors
            qn = [persist.tile([128, N], F16, name=f"qn{m}", tag=f"qn{m}") for m in range(2)]
            kn = [persist.tile([128, N], F16, name=f"kn{m}", tag=f"kn{m}") for m in range(2)]
            y = [
                [
                    persist.tile([128, 512], F16, name=f"y{m}_{qt}", tag=f"y{m}_{qt}")
                    for qt in range(4)
                ]
                for m in range(2)
            ]
            vext = persist.tile([128, NJ, HPC, D + 1], F16, name="vext", tag="vext")
            inv_dram = dramp.tile([8, N], F32, name="inv_dram", tag="inv_dram")
            # softmax-denominator ones column: engine memset
            nc.gpsimd.memset(vext[:, :, :, D : D + 1], 1.0)

            # ---------------- phase 1: projections + norms ----------------
            with (
                tc.tile_pool(name="xw", bufs=1) as xw,
                tc.tile_pool(name="raw", bufs=1) as rawp,
                tc.tile_pool(name="sq", bufs=3) as sqp,
                tc.tile_pool(name="bb", bufs=4) as bbp,
                tc.tile_pool(name="prps", bufs=3, space="PSUM") as prps,
                tc.tile_pool(name="ssps", bufs=2, space="PSUM") as ssps,
            ):
                srt_tm = [
                    [
                        rawp.tile([33, N], F32, name=f"srt{t}{m}", tag=f"srt{t}{m}")
                        for m in range(2)
                    ]
                    for t in range(2)
                ]
                inv_tm = [
                    [
                        rawp.tile([33, N], F32, name=f"inv{t}{m}", tag=f"inv{t}{m}")
                        for m in range(2)
                    ]
                    for t in range(2)
                ]
                # DMA order tuned for earliest first matmul: wq, x[0], wk,
                # x[1..3], wv, wo; x rides [128,1024] half-row tiles so the
                # c-major projection chains can use f=1024 matmuls.
                wq_all = xw.tile([128, NCT, HPC * D], F16, name="wq_all", tag="wq_all")
                nc.scalar.dma_start(
                    wq_all[:], wqT_d[:].rearrange("(c p) d -> p c d", p=128)
                )
                wq_sb = [wq_all[:, c, :] for c in range(NCT)]
                dma_engs = [nc.sync, nc.scalar, nc.gpsimd, nc.sync]
                x2_sb = [[None, None] for _ in range(NCT)]
                for c in range(NCT):
                    for h in range(2):
                        t = xw.tile([128, 1024], F16, name=f"x{c}_{h}", tag=f"x{c}_{h}")
                        dma_engs[(2 * c + h) % 4].dma_start(
                            t[:],
                            x_d[c * 128 : (c + 1) * 128, h * 1024 : (h + 1) * 1024],
                        )
                        x2_sb[c][h] = t
                    if c == 0:
                        wk_all = xw.tile(
                            [128, NCT, HPC * D], F16, name="wk_all", tag="wk_all"
                        )
                        nc.gpsimd.dma_start(
                            wk_all[:], wkT_d[:].rearrange("(c p) d -> p c d", p=128)
                        )
                    if c == 1:
                        wv_all = xw.tile(
                            [128, NCT, HPC * D], F16, name="wv_all", tag="wv_all"
                        )
                        nc.scalar.dma_start(
                            wv_all[:], wvT_d[:].rearrange("(c p) d -> p c d", p=128)
                        )
                wk_sb = [wk_all[:, c, :] for c in range(NCT)]
                wv_sb = [wv_all[:, c, :] for c in range(NCT)]
                for m in range(2):
                    nc.gpsimd.dma_start(
                        wo_sb[m][:], woT_d[m * 128 : (m + 1) * 128, :]
                    )

                # c-major projection: one [128,1024] psum tile per half-row;
                # stationary weights reused across the f=1024 matmuls, so
                # LDWEIGHTS count drops 4x vs nt-major [128,512] chunks.
                # Evac/square emitted right after each group's stop; sumsq
                # matmul + sqrt one group late so the PE never waits on ACT.
                pend_ss = []
                evac_eng = [0]

                def emit_ss(limit):
                    while len(pend_ss) > limit:
                        sq_t, ti_, m_, h_ = pend_ss.pop(0)
                        for q in range(2):
                            ss = ssps.tile([33, 512], F32, name="ss", tag="ss")
                            nc.tensor.matmul(
                                ss[:],
                                lhsT=(ones_w[:]),
                                rhs=(sq_t[:, q * 512 : (q + 1) * 512]),
                                start=True,
                                stop=True,
                            )
                            nc.scalar.activation(
                                srt_tm[ti_][m_][
                                    :, (2 * h_ + q) * 512 : (2 * h_ + q + 1) * 512
                                ],
                                ss[:],
                                AF.Sqrt,
                            )

                def proj_group(m, w_sb, raws, ti):
                    prs = []
                    for h in range(2):
                        prs.append(prps.tile([128, 1024], F32, name="pr", tag="pr"))
                    for c in range(NCT):
                        for h in range(2):
                            for q in range(2):
                                cs = slice(q * 512, (q + 1) * 512)
                                nc.tensor.matmul(
                                    prs[h][:, cs],
                                    lhsT=(w_sb[c][:, m * 128 : (m + 1) * 128]),
                                    rhs=(x2_sb[c][h][:, cs]),
                                    start=(c == 0),
                                    stop=(c == NCT - 1),
                                )
                    for h in range(2):
                        dst = raws[m][:, h * 1024 : (h + 1) * 1024]
                        if evac_eng[0] % 2 == 0:
                            nc.vector.tensor_copy(dst, prs[h][:])
                        else:
                            nc.scalar.activation(dst, prs[h][:], AF.Copy)
                        evac_eng[0] += 1
                        sq = sqp.tile([128, 1024], F32R, name="sq", tag="sq")
                        nc.scalar.activation(sq[:], prs[h][:], AF.Square)
                        pend_ss.append((sq, ti, m, h))

                def norm_head(m):
                    # reciprocals + the inverse-norm row DMA roundtrip with
                    # the 64-partition broadcast.
                    bts = []
                    for ti in range(2):
                        nc.vector.reciprocal_approx_fast(
                            inv_tm[ti][m][:], srt_tm[ti][m][:]
                        )
                        nc.sync.dma_start(
                            inv_dram[4 * ti + 2 * m : 4 * ti + 2 * m + 2, :],
                            inv_tm[ti][m][0:33:32, :],
                        )
                    bt_engs = [nc.sync, nc.gpsimd]
                    for ti in range(2):
                        rowA = 4 * ti + 2 * m
                        bt = bbp.tile([128, N], F32, name="bt", tag="bt")
                        eng = bt_engs[ti]
                        eng.dma_start(
                            bt[0:64, :].unsqueeze(1),
                            inv_dram[rowA : rowA + 1, :].partition_broadcast(64),
                        )
                        eng.dma_start(
                            bt[64:128, :].unsqueeze(1),
                            inv_dram[rowA + 1 : rowA + 2, :].partition_broadcast(64),
                        )
                        bts.append(bt)
                    return bts

                def norm_tail(m, bts):
                    nc.vector.scalar_tensor_tensor(
                        qn[m][:], qn[m][:], qsks8_sb[:], bts[0][:],
                        OP.mult, OP.mult,
                    )
                    nc.gpsimd.tensor_tensor(
                        kn[m][:], kn[m][:], bts[1][:], OP.mult
                    )

                # groups: q0, k0 | q1, k1; each group's sumsq chain runs one
                # group late; norm chains threaded between groups so the
                # inverse-norm DMA roundtrips overlap projection matmuls.
                proj_group(0, wq_sb, qn, 0)
                proj_group(0, wk_sb, kn, 1)
                emit_ss(1)
                proj_group(1, wq_sb, qn, 0)
                emit_ss(1)
                bts0 = norm_head(0)
                proj_group(1, wk_sb, kn, 1)
                emit_ss(1)
                norm_tail(0, bts0)

                # v projection; vext copies alternate ACT/DVE so neither
                # engine is the phase-1 straggler
                for nm_ in range(NJ):
                    psv = prps.tile([128, HPC * D], F32, name="prv", tag="pr")
                    for c in range(NCT):
                        nc.tensor.matmul(
                            psv[:],
                            lhsT=(
                                x2_sb[c][nm_ // 8][
                                    :, (nm_ % 8) * 128 : (nm_ % 8) * 128 + 128
                                ]
                            ),
                            rhs=(wv_sb[c][:]),
                            start=(c == 0),
                            stop=(c == NCT - 1),
                        )
                    if nm_ % 2 == 0:
                        nc.scalar.activation(
                            vext[:, nm_, :, 0:D],
                            psv[:].rearrange("p (h d) -> p h d", h=HPC),
                            AF.Copy,
                        )
                    else:
                        nc.vector.tensor_copy(
                            vext[:, nm_, :, 0:D],
                            psv[:].rearrange("p (h d) -> p h d", h=HPC),
                        )
                    if nm_ == 1:
                        emit_ss(0)
                    if nm_ == 3:
                        bts1 = norm_head(1)
                norm_tail(1, bts1)

            # ---------------- phase 2: attention + fused out-proj ----------
            with (
                tc.tile_pool(name="simps", bufs=4, space="PSUM") as simps,
                tc.tile_pool(name="ops", bufs=3, space="PSUM") as ops,
                tc.tile_pool(name="ppps", bufs=1, space="PSUM") as ppps,
                tc.tile_pool(name="at", bufs=4) as atp,
                tc.tile_pool(name="nrm", bufs=4) as nrm,
                tc.tile_pool(name="fin", bufs=4) as finp,
            ):
                out_engs = [nc.sync, nc.gpsimd, nc.sync, nc.gpsimd]

                def out_proj_halves(qt, ct):
                    qs_ = slice(qt * 512, (qt + 1) * 512)
                    box = {}

                    def start_half():
                        pp = ppps.tile([128, 512], F32, name="pp", tag="pp")
                        box["pp"] = pp
                        nc.tensor.matmul(
                            pp[:],
                            lhsT=(wo_sb[0][:, ct * 128 : (ct + 1) * 128]),
                            rhs=(y[0][qt][:]),
                            start=True,
                            stop=False,
                        )
                        # bias folded into the accumulation: bias_row x ones
                        nc.tensor.matmul(
                            pp[:],
                            lhsT=(biasr_sb[:, ct * 128 : (ct + 1) * 128]),
                            rhs=(ones512[:]),
                            start=False,
                            stop=False,
                        )

                    def finish_half():
                        pp = box["pp"]
                        nc.tensor.matmul(
                            pp[:],
                            lhsT=(wo_sb[1][:, ct * 128 : (ct + 1) * 128]),
                            rhs=(y[1][qt][:]),
                            start=False,
                            stop=True,
                        )
                        ot = finp.tile([128, 512], F32, name="ot", tag="ot")
                        # evac engines alternate so one stalled copy can't
                        # block the other engine's exp stream
                        if ct % 2 == 0:
                            nc.scalar.activation(ot[:], pp[:], AF.Copy)
                        else:
                            nc.vector.tensor_copy(ot[:], pp[:])
                        out_engs[ct].dma_start(
                            out_d[ct * 128 : (ct + 1) * 128, qs_], ot[:]
                        )

                    return start_half, finish_half

                def av_pair(at_t, j_, oA, oB, hA, hB):
                    nc.tensor.matmul(
                        oA[:],
                        lhsT=(vext[:, j_, hA, :]),
                        rhs=(at_t[:, 0:512]),
                        start=(j_ == 0),
                        stop=(j_ == NJ - 1),
                    )
                    nc.tensor.matmul(
                        oB[:],
                        lhsT=(vext[:, j_, hB, :]),
                        rhs=(at_t[:, 512:1024]),
                        start=(j_ == 0),
                        stop=(j_ == NJ - 1),
                    )

                def normalize(oA, oB, m, qt):
                    # ACT evacuates the accumulator (rows 0:64) and the
                    # denominator row separately (offset-64 PSUM reads are
                    # only correct on ACT); DVE recips; gpsimd broadcasts
                    # and multiplies into fp16 y (gpsimd cannot touch PSUM).
                    for o_ps, base in ((oA, 0), (oB, 64)):
                        # denominator row first so the recip/broadcast chain
                        # starts before the bulk evacuation
                        rsb = nrm.tile([1, 512], F32, name="rsb", tag="rsb")
                        nc.scalar.activation(rsb[:], o_ps[D : D + 1, :], AF.Copy)
                        inv = nrm.tile([1, 512], F32, name="inv", tag="inv")
                        nc.vector.reciprocal_approx_fast(inv[:], rsb[:])
                        oc = nrm.tile([64, 512], F32, name="oc", tag="oc")
                        nc.scalar.activation(oc[:], o_ps[0:D, :], AF.Copy)
                        br = nrm.tile([64, 512], F32, name="br", tag="br")
                        nc.gpsimd.partition_broadcast(br[:], inv[:], channels=64)
                        nc.gpsimd.tensor_tensor(
                            y[m][qt][base : base + 64, :],
                            oc[:],
                            br[:],
                            OP.mult,
                        )

                pend_av = [None]
                pend_pp = []
                astep = [0]

                def flush_pend(last=False):
                    if pend_av[0] is None:
                        return
                    at_t, j_, oA, oB, m_, qt_ = pend_av[0]
                    pend_av[0] = None
                    av_pair(at_t, j_, oA, oB, 2 * m_, 2 * m_ + 1)
                    if j_ == NJ - 1:
                        normalize(oA, oB, m_, qt_)
                        if m_ == 1:
                            for ct in range(NCT):
                                for fn in out_proj_halves(qt_, ct):
                                    if last:
                                        fn()
                                    else:
                                        pend_pp.append((astep[0] + 8, fn))

                for qt in range(NQT):
                    for m in range(2):
                        qs_ = slice(qt * 512, (qt + 1) * 512)
                        oA = ops.tile([D + 1, 512], F32, name="oA", tag="o")
                        oB = ops.tile([D + 1, 512], F32, name="oB", tag="o")
                        for j in range(NJ):
                            js = slice(j * 128, (j + 1) * 128)
                            simA = simps.tile([128, 512], F32, name="simA", tag="sim")
                            simB = simps.tile([128, 512], F32, name="simB", tag="sim")
                            nc.tensor.matmul(
                                simA[:],
                                lhsT=(kn[m][0:64, js]),
                                rhs=(qn[m][0:64, qs_]),
                                start=True,
                                stop=True,
                            )
                            nc.tensor.matmul(
                                simB[:],
                                lhsT=(kn[m][64:128, js]),
                                rhs=(qn[m][64:128, qs_]),
                                start=True,
                                stop=True,
                            )
                            flush_pend()
                            if pend_pp and j % 2 == 1 and astep[0] >= pend_pp[0][0]:
                                pend_pp.pop(0)[1]()
                            at = atp.tile([128, 1024], F16, name="at", tag="at")
                            # exact exp on ACT for head A
                            nc.scalar.activation(at[:, 0:512], simA[:], AF.Exp)
                            # Schraudolph fast-exp on DVE for head B
                            nc.vector.tensor_scalar(
                                at[:, 512:1024].bitcast(I16),
                                simB[:],
                                S_SCH,
                                B_SCH,
                                op0=OP.mult,
                                op1=OP.add,
                            )
                            pend_av[0] = (at, j, oA, oB, m, qt)
                            astep[0] += 1
                flush_pend(last=True)
                while pend_pp:
                    pend_pp.pop(0)[1]()

    nc.finalize()
    return nc


def kernel(x, Wqkv, q_scale, k_scale, Wout, bout):
    global _CACHED_NC, LAST_RESULTS
    x = np.asarray(x, dtype=np.float32)
    Wqkv = np.asarray(Wqkv, dtype=np.float32)
    q_scale = np.asarray(q_scale, dtype=np.float32)
    k_scale = np.asarray(k_scale, dtype=np.float32)
    Wout = np.asarray(Wout, dtype=np.float32)
    bout = np.asarray(bout, dtype=np.float32)

    if _CACHED_NC is None:
        _CACHED_NC = build_nc()
    nc = _CACHED_NC

    H_DIM = HEADS * D
    qsks8 = np.tile((SCALE * q_scale * k_scale).astype(np.float32), 2)[:, None]
    qsks8 = np.ascontiguousarray(qsks8)
    biasr = np.ascontiguousarray((bout / 4.0).astype(np.float16)[None, :])
    onesw = np.zeros((128, 33), dtype=np.float32)
    onesw[0:64, 0] = 1.0
    onesw[64:128, 32] = 1.0

    in_maps = []
    for core in range(NCORES):
        b = core // 4
        h0 = HPC * (core % 4)
        rs = slice(h0 * D, h0 * D + HPC * D)
        wq = Wqkv[0:H_DIM][rs]
        wk = Wqkv[H_DIM : 2 * H_DIM][rs]
        wv = Wqkv[2 * H_DIM : 3 * H_DIM][rs]
        in_maps.append(
            {
                "x": np.ascontiguousarray(x[b]).astype(np.float16),
                "wqT": np.ascontiguousarray(wq.T).astype(np.float16),
                "wkT": np.ascontiguousarray(wk.T).astype(np.float16),
                "wvT": np.ascontiguousarray(wv.T).astype(np.float16),
                "woT": np.ascontiguousarray(Wout[:, rs].T).astype(np.float16),
                "qsks8": qsks8,
                "onesw": onesw,
                "biasr": biasr,
            }
        )

    res = run_bass_kernel_spmd(
        nc,
        in_maps,
        core_ids=list(range(NCORES)),
        trace=bool(os.environ.get("BASS_TRACE")),
        **EXTRA_RUN_KWARGS,
    )
    LAST_RESULTS = res

    outs = [np.asarray(res.results[i]["out"], dtype=np.float32) for i in range(NCORES)]
    full = np.empty((B, C, N), dtype=np.float32)
    full[0] = outs[0] + outs[1] + outs[2] + outs[3]
    full[1] = outs[4] + outs[5] + outs[6] + outs[7]
    return full


# revision 16
# speedup vs baseline: 1.6000x; 1.5833x over previous
"""Trainium2 Bass kernel for cosine-similarity ("sparse") attention.

Reference computation (B=2, C=512, N=2048, H=16, D=64, SCALE=8):
    qkv = Wqkv @ x                          # 1x1 conv
    q,k,v -> [B,H,D,N]
    q = l2norm(q, over D) * q_scale ; k = l2norm(k, over D) * k_scale
    sim = (q^T k) * 8 ; attn = softmax(sim, over keys)
    out = Wout @ (attn @ v) + bout

Sharding: 32 (batch, head) pairs across 8 cores -> each core owns one batch
(b = core//4) and 4 heads (h0 = 4*(core%4)).  Each core projects q/k/v for
its heads, runs attention, and computes a partial output projection
Wout[:, its-heads] @ y + bout/4.  Host sums the 4 partials per batch.

Device-side schedule (per core):
  - fp16 for every 16-bit tensor (x, W, qn/kn, vext, at, y, wo): fp16's 10
    mantissa bits beat bf16 ~8x on this problem, which buys the error
    budget for the fast-exp trick below.
  - The [128,1024] softmax exp (2 heads x 512 queries per j-step) is
    split per head: ACT does head A with the exact Exp table; DVE does
    head B with a Schraudolph bit-trick exp: i16 = round(sim*1024*log2e
    + (15*1024 - 44)) via one tensor_scalar (fp32 PSUM -> int16), the
    int16 tile being the fp16 `at` tile bitcast.  ~+-4% on those columns,
    but softmax self-normalisation cancels the common mode; end-to-end
    rel err ~8e-3 (gate 2e-2).
  - sim is two per-head [128,512] PSUM tiles (bufs=4) so the engines
    never share a tile and block handoffs don't stall on bank rotation.
  - Softmax denominator: ones column in vext -> row D of the av
    accumulator.  Chain: ACT Copy evacuates rows [0:64] and row 64 (ACT
    handles partition-offset PSUM reads; custom-DVE ops and gpsimd
    partition_broadcast do NOT -- verified on HW), DVE recip, gpsimd
    broadcast, gpsimd multiply into fp16 y (gpsimd has no PSUM port).
  - out-proj bias rides the PSUM accumulation as a third matmul
    (bias_row x ones_row), so the evacuation is a plain ACT Copy (Copy
    shares the exp table set -- no ACT table thrash).
  - The av pair runs one j-step behind the sim pair ACROSS block
    boundaries; out-proj column chunks are staggered one per 2 j-steps;
    the last block flushes immediately to shorten the tail.
  - Softmax max-subtraction is skipped: sim = 8*cosine is in [-8, 8]
    (which also keeps the Schraudolph i16 strictly positive).
  - l2norm: ones-indicator matmul for sumsq, fused PSUM->SBUF Sqrt on
    ACT, DVE reciprocal, inverse-norm broadcast via a DMA roundtrip.
  - Startup: x rides one DMA per 128-partition chunk; wq first so the
    first projection starts as early as possible.
"""

import os
import sys

import numpy as np

sys.path.insert(0, "/opt/trn_rl_repo")

import concourse.bass as bass  # noqa: E402
import concourse.mybir as mybir  # noqa: E402
from concourse import bacc, tile  # noqa: E402
from concourse.bass_utils import run_bass_kernel_spmd  # noqa: E402

F32 = mybir.dt.float32
F32R = mybir.dt.float32r
F16 = mybir.dt.float16
I16 = mybir.dt.int16
AF = mybir.ActivationFunctionType
OP = mybir.AluOpType

B, C, N = 2, 512, 2048
HEADS, D = 16, 64
SCALE = 8.0
NCORES = 8
HPC = 4  # heads per core

# Schraudolph fast-exp constants (fp16 bit layout):
# i16 = round(sim * S_SCH + B_SCH); bits reinterpreted as fp16 ~= e^sim.
S_SCH = 1024.0 / float(np.log(2.0))
B_SCH = 15.0 * 1024.0 - 44.0

_CACHED_NC = None
LAST_RESULTS = None
EXTRA_RUN_KWARGS = {}


def build_nc():
    nc = bacc.Bacc(None, target_bir_lowering=False)

    x_d = nc.declare_dram_parameter("x", [C, N], F16, isOutput=False)
    wqT_d = nc.declare_dram_parameter("wqT", [C, HPC * D], F16, isOutput=False)
    wkT_d = nc.declare_dram_parameter("wkT", [C, HPC * D], F16, isOutput=False)
    wvT_d = nc.declare_dram_parameter("wvT", [C, HPC * D], F16, isOutput=False)
    woT_d = nc.declare_dram_parameter("woT", [HPC * D, C], F16, isOutput=False)
    qsks8_d = nc.declare_dram_parameter("qsks8", [128, 1], F32, isOutput=False)
    onesw_d = nc.declare_dram_parameter("onesw", [128, 33], F32R, isOutput=False)
    ones64r_d = nc.declare_dram_parameter("ones64r", [1, 64], F32R, isOutput=False)
    biasr_d = nc.declare_dram_parameter("biasr", [1, C], F16, isOutput=False)
    out_d = nc.declare_dram_parameter("out", [C, N], F32, isOutput=True)

    NQT = N // 512  # 4 query chunks of 512
    NJ = N // 128  # 16 key chunks of 128
    NCT = C // 128  # 4 channel chunks of 128

    with tile.TileContext(nc) as tc:
        with (
            tc.tile_pool(name="const", bufs=1) as const,
            tc.tile_pool(name="persist", bufs=1) as persist,
            tc.tile_pool(name="dramp", bufs=1, space="DRAM") as dramp,
        ):
            qsks8_sb = const.tile([128, 1], F32, name="qsks8", tag="qsks8")
            nc.sync.dma_start(qsks8_sb[:], qsks8_d[:])
            biasr_sb = const.tile([1, C], F16, name="biasr", tag="biasr")
            nc.sync.dma_start(biasr_sb[:], biasr_d[:])
            ones512 = const.tile([1, 512], F16, name="ones512", tag="ones512")
            nc.gpsimd.memset(ones512[:], 1.0)
            # indicator weights: col 0 sums partitions 0-63 (head A), col 32
            # sums partitions 64-127 (head B); middle cols write zeros so the
            # [33, 512] sumsq psum rows land 32-aligned.
            ones_w = const.tile([128, 33], F32R, name="ones_w", tag="ones_w")
            nc.sync.dma_start(ones_w[:], onesw_d[:])
            ones64r = const.tile([1, 64], F32R, name="ones64r", tag="ones64r")
            nc.sync.dma_start(ones64r[:], ones64r_d[:])
            wo_sb = [
                const.tile([128, C], F16, name=f"wo{m}", tag=f"wo{m}")
                for m in range(2)
            ]

            # persistent tensors
            qn = [persist.tile([128, N], F16, name=f"qn{m}", tag=f"qn{m}") for m in range(2)]
            kn = [persist.tile([128, N], F16, name=f"kn{m}", tag=f"kn{m}") for m in range(2)]
            y = [
                [
                    persist.tile([128, 512], F16, name=f"y{m}_{qt}", tag=f"y{m}_{qt}")
                    for qt in range(4)
                ]
                for m in range(2)
            ]
            vext = persist.tile([128, NJ, HPC, D + 1], F16, name="vext", tag="vext")
            inv_dram = dramp.tile([8, N], F32, name="inv_dram", tag="inv_dram")
            # softmax-denominator ones column: engine memset
            nc.gpsimd.memset(vext[:, :, :, D : D + 1], 1.0)

            # ---------------- phase 1: projections + norms ----------------
            with (
                tc.tile_pool(name="xw", bufs=1) as xw,
                tc.tile_pool(name="raw", bufs=1) as rawp,
                tc.tile_pool(name="sq", bufs=3) as sqp,
                tc.tile_pool(name="bb", bufs=4) as bbp,
                tc.tile_pool(name="prps", bufs=3, space="PSUM") as prps,
                tc.tile_pool(name="ssps", bufs=2, space="PSUM") as ssps,
            ):
                srt_tm = [
                    [
                        rawp.tile([33, N], F32, name=f"srt{t}{m}", tag=f"srt{t}{m}")
                        for m in range(2)
                    ]
                    for t in range(2)
                ]
                inv_tm = [
                    [
                        rawp.tile([33, N], F32, name=f"inv{t}{m}", tag=f"inv{t}{m}")
                        for m in range(2)
                    ]
                    for t in range(2)
                ]
                # DMA order tuned for earliest first matmul: wq, x[0], wk,
                # x[1..3], wv, wo; x rides [128,1024] half-row tiles so the
                # c-major projection chains can use f=1024 matmuls.
                wq_all = xw.tile([128, NCT, HPC * D], F16, name="wq_all", tag="wq_all")
                nc.scalar.dma_start(
                    wq_all[:], wqT_d[:].rearrange("(c p) d -> p c d", p=128)
                )
                wq_sb = [wq_all[:, c, :] for c in range(NCT)]
                dma_engs = [nc.sync, nc.scalar, nc.gpsimd, nc.sync]
                x2_sb = [[None, None] for _ in range(NCT)]
                for c in range(NCT):
                    for h in range(2):
                        t = xw.tile([128, 1024], F16, name=f"x{c}_{h}", tag=f"x{c}_{h}")
                        dma_engs[(2 * c + h) % 4].dma_start(
                            t[:],
                            x_d[c * 128 : (c + 1) * 128, h * 1024 : (h + 1) * 1024],
                        )
                        x2_sb[c][h] = t
                    if c == 0:
                        wk_all = xw.tile(
                            [128, NCT, HPC * D], F16, name="wk_all", tag="wk_all"
                        )
                        nc.gpsimd.dma_start(
                            wk_all[:], wkT_d[:].rearrange("(c p) d -> p c d", p=128)
                        )
                    if c == 1:
                        wv_all = xw.tile(
                            [128, NCT, HPC * D], F16, name="wv_all", tag="wv_all"
                        )
                        nc.scalar.dma_start(
                            wv_all[:], wvT_d[:].rearrange("(c p) d -> p c d", p=128)
                        )
                wk_sb = [wk_all[:, c, :] for c in range(NCT)]
                wv_sb = [wv_all[:, c, :] for c in range(NCT)]
                for m in range(2):
                    nc.gpsimd.dma_start(
                        wo_sb[m][:], woT_d[m * 128 : (m + 1) * 128, :]
                    )

                # c-major projection: one [128,1024] psum tile per half-row;
                # stationary weights reused across the f=1024 matmuls, so
                # LDWEIGHTS count drops 4x vs nt-major [128,512] chunks.
                # Evac/square emitted right after each group's stop; sumsq
                # matmul + sqrt one group late so the PE never waits on ACT.
                pend_ss = []
                evac_eng = [0]

                def emit_ss(limit):
                    while len(pend_ss) > limit:
                        sq_t, ti_, m_, h_ = pend_ss.pop(0)
                        for q in range(2):
                            ss = ssps.tile([33, 512], F32, name="ss", tag="ss")
                            nc.tensor.matmul(
                                ss[:],
                                lhsT=(ones_w[:]),
                                rhs=(sq_t[:, q * 512 : (q + 1) * 512]),
                                start=True,
                                stop=True,
                            )
                            nc.scalar.activation(
                                srt_tm[ti_][m_][
                                    :, (2 * h_ + q) * 512 : (2 * h_ + q + 1) * 512
                                ],
                                ss[:],
                                AF.Sqrt,
                            )

                def proj_group(m, w_sb, raws, ti):
                    prs = []
                    for h in range(2):
                        prs.append(prps.tile([128, 1024], F32, name="pr", tag="pr"))
                    for c in range(NCT):
                        for h in range(2):
                            for q in range(2):
                                cs = slice(q * 512, (q + 1) * 512)
                                nc.tensor.matmul(
                                    prs[h][:, cs],
                                    lhsT=(w_sb[c][:, m * 128 : (m + 1) * 128]),
                                    rhs=(x2_sb[c][h][:, cs]),
                                    start=(c == 0),
                                    stop=(c == NCT - 1),
                                )
                    # q evacuations fold the qsks8 per-partition scale in
                    # (free on both engines); k evacuations are plain copies
                    scaled = raws is qn
                    for h in range(2):
                        dst = raws[m][:, h * 1024 : (h + 1) * 1024]
                        if evac_eng[0] % 2 == 0:
                            if scaled:
                                nc.vector.tensor_scalar(
                                    dst, prs[h][:], qsks8_sb[:], None, op0=OP.mult
                                )
                            else:
                                nc.vector.tensor_copy(dst, prs[h][:])
                        else:
                            nc.scalar.activation(
                                dst, prs[h][:], AF.Copy,
                                scale=qsks8_sb[:] if scaled else 1.0,
                            )
                        evac_eng[0] += 1
                        sq = sqp.tile([128, 1024], F32R, name="sq", tag="sq")
                        nc.scalar.activation(sq[:], prs[h][:], AF.Square)
                        pend_ss.append((sq, ti, m, h))

                def norm_head(m):
                    # reciprocals + the inverse-norm row DMA roundtrip with
                    # the 64-partition broadcast.
                    bts = []
                    for ti in range(2):
                        nc.vector.reciprocal_approx_fast(
                            inv_tm[ti][m][:], srt_tm[ti][m][:]
                        )
                        nc.sync.dma_start(
                            inv_dram[4 * ti + 2 * m : 4 * ti + 2 * m + 2, :],
                            inv_tm[ti][m][0:33:32, :],
                        )
                    bt_engs = [nc.sync, nc.gpsimd]
                    for ti in range(2):
                        rowA = 4 * ti + 2 * m
                        bt = bbp.tile([128, N], F32, name="bt", tag="bt")
                        eng = bt_engs[ti]
                        eng.dma_start(
                            bt[0:64, :].unsqueeze(1),
                            inv_dram[rowA : rowA + 1, :].partition_broadcast(64),
                        )
                        eng.dma_start(
                            bt[64:128, :].unsqueeze(1),
                            inv_dram[rowA + 1 : rowA + 2, :].partition_broadcast(64),
                        )
                        bts.append(bt)
                    return bts

                def norm_tail(m, bts):
                    # qsks8 already folded in at evacuation time; plain
                    # inverse-norm multiplies, both on DVE (3-operand stt
                    # runs at half DVE rate; gpsimd pays ~6us/op turnaround)
                    nc.vector.tensor_tensor(
                        qn[m][:], qn[m][:], bts[0][:], OP.mult
                    )
                    nc.vector.tensor_tensor(
                        kn[m][:], kn[m][:], bts[1][:], OP.mult
                    )

                # groups: q0, k0 | q1, k1; each group's sumsq chain runs one
                # group late; norm chains threaded between groups so the
                # inverse-norm DMA roundtrips overlap projection matmuls.
                proj_group(0, wq_sb, qn, 0)
                proj_group(0, wk_sb, kn, 1)
                emit_ss(1)
                proj_group(1, wq_sb, qn, 0)
                emit_ss(1)
                bts0 = norm_head(0)
                proj_group(1, wk_sb, kn, 1)
                emit_ss(1)
                norm_tail(0, bts0)

                # v projection; vext copies alternate ACT/DVE so neither
                # engine is the phase-1 straggler
                for nm_ in range(NJ):
                    psv = prps.tile([128, HPC * D], F32, name="prv", tag="pr")
                    for c in range(NCT):
                        nc.tensor.matmul(
                            psv[:],
                            lhsT=(
                                x2_sb[c][nm_ // 8][
                                    :, (nm_ % 8) * 128 : (nm_ % 8) * 128 + 128
                                ]
                            ),
                            rhs=(wv_sb[c][:]),
                            start=(c == 0),
                            stop=(c == NCT - 1),
                        )
                    if nm_ % 2 == 0:
                        nc.scalar.activation(
                            vext[:, nm_, :, 0:D],
                            psv[:].rearrange("p (h d) -> p h d", h=HPC),
                            AF.Copy,
                        )
                    else:
                        nc.vector.tensor_copy(
                            vext[:, nm_, :, 0:D],
                            psv[:].rearrange("p (h d) -> p h d", h=HPC),
                        )
                    if nm_ == 1:
                        emit_ss(0)
                    if nm_ == 3:
                        bts1 = norm_head(1)
                norm_tail(1, bts1)

            # ---------------- phase 2: attention + fused out-proj ----------
            # PSUM banks: sim 3 + o 4 + (pp|br shared) 1 = 8.  No gpsimd op
            # anywhere in this phase (each Q7 software op costs ~6us of
            # turnaround, which serialized the whole block handoff).
            with (
                tc.tile_pool(name="simps", bufs=3, space="PSUM") as simps,
                tc.tile_pool(name="ops", bufs=4, space="PSUM") as ops,
                tc.tile_pool(name="ppps", bufs=1, space="PSUM") as ppps,
                tc.tile_pool(name="at", bufs=4) as atp,
                tc.tile_pool(name="nrm", bufs=4) as nrm,
                tc.tile_pool(name="fin", bufs=4) as finp,
            ):
                def out_proj_halves(qt, ct):
                    qs_ = slice(qt * 512, (qt + 1) * 512)
                    box = {}

                    def start_half():
                        pp = ppps.tile([128, 512], F32, name="pp", tag="pp")
                        box["pp"] = pp
                        nc.tensor.matmul(
                            pp[:],
                            lhsT=(wo_sb[0][:, ct * 128 : (ct + 1) * 128]),
                            rhs=(y[0][qt][:]),
                            start=True,
                            stop=False,
                        )
                        # bias folded into the accumulation: bias_row x ones
                        nc.tensor.matmul(
                            pp[:],
                            lhsT=(biasr_sb[:, ct * 128 : (ct + 1) * 128]),
                            rhs=(ones512[:]),
                            start=False,
                            stop=False,
                        )

                    def finish_half():
                        pp = box["pp"]
                        nc.tensor.matmul(
                            pp[:],
                            lhsT=(wo_sb[1][:, ct * 128 : (ct + 1) * 128]),
                            rhs=(y[1][qt][:]),
                            start=False,
                            stop=True,
                        )
                        ot = finp.tile([128, 512], F32, name="ot", tag="ot")
                        nc.scalar.activation(ot[:], pp[:], AF.Copy)
                        nc.sync.dma_start(
                            out_d[ct * 128 : (ct + 1) * 128, qs_], ot[:]
                        )

                    return start_half, finish_half

                def av_pair(at_t, j_, oA, oB, hA, hB):
                    nc.tensor.matmul(
                        oA[:],
                        lhsT=(vext[:, j_, hA, :]),
                        rhs=(at_t[:, 0:512]),
                        start=(j_ == 0),
                        stop=(j_ == NJ - 1),
                    )
                    nc.tensor.matmul(
                        oB[:],
                        lhsT=(vext[:, j_, hB, :]),
                        rhs=(at_t[:, 512:1024]),
                        start=(j_ == 0),
                        stop=(j_ == NJ - 1),
                    )

                def normalize_stage1(oA, oB):
                    # ACT evacuates the denominator rows straight off the
                    # PSUM accumulators into F32R (offset-64 PSUM reads are
                    # only correct on ACT standard ops)
                    rs = []
                    for o_ps in (oA, oB):
                        rsb = nrm.tile([1, 512], F32R, name="rsb", tag="rsb")
                        nc.scalar.activation(rsb[:], o_ps[D : D + 1, :], AF.Copy)
                        rs.append(rsb)
                    return rs

                def normalize_stage2(oA, oB, rs, m, qt):
                    # PE broadcasts the raw denominator row (ones64r f32r
                    # matmul, 1 bank shared with the out-proj pool), DVE
                    # recips the [64,512] and multiplies the accumulator
                    # into fp16 y -- one PSUM operand per DVE op.
                    for o_ps, rsb, base in ((oA, rs[0], 0), (oB, rs[1], 64)):
                        br = ppps.tile([64, 512], F32, name="br", tag="pp")
                        nc.tensor.matmul(
                            br[:], lhsT=(ones64r[:]), rhs=(rsb[:]),
                            start=True, stop=True,
                        )
                        bri = nrm.tile([64, 512], F32, name="bri", tag="bri")
                        nc.vector.reciprocal_approx_fast(bri[:], br[:])
                        nc.vector.tensor_tensor(
                            y[m][qt][base : base + 64, :],
                            o_ps[0:D, :],
                            bri[:],
                            OP.mult,
                        )

                pend_av = [None]
                norm_pend = []
                pend_pp = []
                astep = [0]
                block_o = {}

                def flush_pend(last=False):
                    # stage2 of the previous block's normalize goes first so
                    # its br allocation precedes this block's pp allocations
                    while norm_pend and (last or norm_pend[0][0] <= astep[0]):
                        norm_pend.pop(0)[1]()
                    if pend_av[0] is None:
                        return
                    at_t, j_, m_, qt_ = pend_av[0]
                    pend_av[0] = None
                    if j_ == 0:
                        block_o[(m_, qt_)] = (
                            ops.tile([D + 1, 512], F32, name="oA", tag="o"),
                            ops.tile([D + 1, 512], F32, name="oB", tag="o"),
                        )
                    oA, oB = block_o[(m_, qt_)]
                    av_pair(at_t, j_, oA, oB, 2 * m_, 2 * m_ + 1)
                    if j_ == NJ - 1:
                        rs = normalize_stage1(oA, oB)
                        norm_pend.append(
                            (
                                astep[0] + 1,
                                lambda oA=oA, oB=oB, rs=rs, m_=m_, qt_=qt_: (
                                    normalize_stage2(oA, oB, rs, m_, qt_)
                                ),
                            )
                        )
                        if m_ == 1:
                            for ct in range(NCT):
                                for fn in out_proj_halves(qt_, ct):
                                    if last:
                                        pend_pp.append((0, fn))
                                    else:
                                        pend_pp.append((astep[0] + 5, fn))

                for qt in range(NQT):
                    for m in range(2):
                        qs_ = slice(qt * 512, (qt + 1) * 512)
                        for j in range(NJ):
                            js = slice(j * 128, (j + 1) * 128)
                            simA = simps.tile([128, 512], F32, name="simA", tag="sim")
                            simB = simps.tile([128, 512], F32, name="simB", tag="sim")
                            nc.tensor.matmul(
                                simA[:],
                                lhsT=(kn[m][0:64, js]),
                                rhs=(qn[m][0:64, qs_]),
                                start=True,
                                stop=True,
                            )
                            nc.tensor.matmul(
                                simB[:],
                                lhsT=(kn[m][64:128, js]),
                                rhs=(qn[m][64:128, qs_]),
                                start=True,
                                stop=True,
                            )
                            flush_pend()
                            if pend_pp and astep[0] >= pend_pp[0][0]:
                                pend_pp.pop(0)[1]()
                            at = atp.tile([128, 1024], F16, name="at", tag="at")
                            # exact exp on ACT for head A
                            nc.scalar.activation(at[:, 0:512], simA[:], AF.Exp)
                            # Schraudolph fast-exp on DVE for head B
                            nc.vector.tensor_scalar(
                                at[:, 512:1024].bitcast(I16),
                                simB[:],
                                S_SCH,
                                B_SCH,
                                op0=OP.mult,
                                op1=OP.add,
                            )
                            pend_av[0] = (at, j, m, qt)
                            astep[0] += 1
                flush_pend(last=True)
                while norm_pend:
                    norm_pend.pop(0)[1]()
                while pend_pp:
                    pend_pp.pop(0)[1]()

    nc.finalize()
    return nc


def kernel(x, Wqkv, q_scale, k_scale, Wout, bout):
    global _CACHED_NC, LAST_RESULTS
    x = np.asarray(x, dtype=np.float32)
    Wqkv = np.asarray(Wqkv, dtype=np.float32)
    q_scale = np.asarray(q_scale, dtype=np.float32)
    k_scale = np.asarray(k_scale, dtype=np.float32)
    Wout = np.asarray(Wout, dtype=np.float32)
    bout = np.asarray(bout, dtype=np.float32)

    if _CACHED_NC is None:
        _CACHED_NC = build_nc()
    nc = _CACHED_NC

    H_DIM = HEADS * D
    qsks8 = np.tile((SCALE * q_scale * k_scale).astype(np.float32), 2)[:, None]
    qsks8 = np.ascontiguousarray(qsks8)
    biasr = np.ascontiguousarray((bout / 4.0).astype(np.float16)[None, :])
    onesw = np.zeros((128, 33), dtype=np.float32)
    onesw[0:64, 0] = 1.0
    onesw[64:128, 32] = 1.0

    in_maps = []
    for core in range(NCORES):
        b = core // 4
        h0 = HPC * (core % 4)
        rs = slice(h0 * D, h0 * D + HPC * D)
        wq = Wqkv[0:H_DIM][rs]
        wk = Wqkv[H_DIM : 2 * H_DIM][rs]
        wv = Wqkv[2 * H_DIM : 3 * H_DIM][rs]
        in_maps.append(
            {
                "x": np.ascontiguousarray(x[b]).astype(np.float16),
                "wqT": np.ascontiguousarray(wq.T).astype(np.float16),
                "wkT": np.ascontiguousarray(wk.T).astype(np.float16),
                "wvT": np.ascontiguousarray(wv.T).astype(np.float16),
                "woT": np.ascontiguousarray(Wout[:, rs].T).astype(np.float16),
                "qsks8": qsks8,
                "onesw": onesw,
                "ones64r": np.ones((1, 64), dtype=np.float32),
                "biasr": biasr,
            }
        )

    res = run_bass_kernel_spmd(
        nc,
        in_maps,
        core_ids=list(range(NCORES)),
        trace=bool(os.environ.get("BASS_TRACE")),
        **EXTRA_RUN_KWARGS,
    )
    LAST_RESULTS = res

    outs = [np.asarray(res.results[i]["out"], dtype=np.float32) for i in range(NCORES)]
    full = np.empty((B, C, N), dtype=np.float32)
    full[0] = outs[0] + outs[1] + outs[2] + outs[3]
    full[1] = outs[4] + outs[5] + outs[6] + outs[7]
    return full


# revision 28
# speedup vs baseline: 1.6723x; 1.0451x over previous
"""Trainium2 Bass kernel for cosine-similarity ("sparse") attention.

Reference computation (B=2, C=512, N=2048, H=16, D=64, SCALE=8):
    qkv = Wqkv @ x                          # 1x1 conv
    q,k,v -> [B,H,D,N]
    q = l2norm(q, over D) * q_scale ; k = l2norm(k, over D) * k_scale
    sim = (q^T k) * 8 ; attn = softmax(sim, over keys)
    out = Wout @ (attn @ v) + bout

Sharding: 32 (batch, head) pairs across 8 cores -> each core owns one batch
(b = core//4) and 4 heads (h0 = 4*(core%4)).  Each core projects q/k/v for
its heads, runs attention, and computes a partial output projection
Wout[:, its-heads] @ y + bout/4.  Host sums the 4 partials per batch.

Device-side schedule (per core):
  - fp16 for every 16-bit tensor (x, W, qn/kn, vext, at, y, wo): fp16's 10
    mantissa bits beat bf16 ~8x on this problem, which buys the error
    budget for the fast-exp trick below.
  - The [128,1024] softmax exp (2 heads x 512 queries per j-step) is
    split per head: ACT does head A with the exact Exp table; DVE does
    head B with a Schraudolph bit-trick exp: i16 = round(sim*1024*log2e
    + (15*1024 - 44)) via one tensor_scalar (fp32 PSUM -> int16), the
    int16 tile being the fp16 `at` tile bitcast.  ~+-4% on those columns,
    but softmax self-normalisation cancels the common mode; end-to-end
    rel err ~8e-3 (gate 2e-2).
  - sim is two per-head [128,512] PSUM tiles (bufs=4) so the engines
    never share a tile and block handoffs don't stall on bank rotation.
  - Softmax denominator: ones column in vext -> row D of the av
    accumulator.  Chain: ACT Copy evacuates rows [0:64] and row 64 (ACT
    handles partition-offset PSUM reads; custom-DVE ops and gpsimd
    partition_broadcast do NOT -- verified on HW), DVE recip, gpsimd
    broadcast, gpsimd multiply into fp16 y (gpsimd has no PSUM port).
  - out-proj bias rides the PSUM accumulation as a third matmul
    (bias_row x ones_row), so the evacuation is a plain ACT Copy (Copy
    shares the exp table set -- no ACT table thrash).
  - The av pair runs one j-step behind the sim pair ACROSS block
    boundaries; out-proj column chunks are staggered one per 2 j-steps;
    the last block flushes immediately to shorten the tail.
  - Softmax max-subtraction is skipped: sim = 8*cosine is in [-8, 8]
    (which also keeps the Schraudolph i16 strictly positive).
  - l2norm: ones-indicator matmul for sumsq, fused PSUM->SBUF Sqrt on
    ACT, DVE reciprocal, inverse-norm broadcast via a DMA roundtrip.
  - Startup: x rides one DMA per 128-partition chunk; wq first so the
    first projection starts as early as possible.
"""

import os
import sys

import numpy as np

sys.path.insert(0, "/opt/trn_rl_repo")

import concourse.bass as bass  # noqa: E402
import concourse.mybir as mybir  # noqa: E402
from concourse import bacc, tile  # noqa: E402
from concourse.bass_utils import run_bass_kernel_spmd  # noqa: E402

F32 = mybir.dt.float32
F32R = mybir.dt.float32r
F16 = mybir.dt.float16
I16 = mybir.dt.int16
AF = mybir.ActivationFunctionType
OP = mybir.AluOpType

B, C, N = 2, 512, 2048
HEADS, D = 16, 64
SCALE = 8.0
NCORES = 8
HPC = 4  # heads per core

# Schraudolph fast-exp constants (fp16 bit layout):
# i16 = round(sim * S_SCH + B_SCH); bits reinterpreted as fp16 ~= e^sim.
S_SCH = 1024.0 / float(np.log(2.0))
B_SCH = 15.0 * 1024.0 - 44.0

_CACHED_NC = None
LAST_RESULTS = None
EXTRA_RUN_KWARGS = {}


def build_nc():
    nc = bacc.Bacc(None, target_bir_lowering=False)

    x_d = nc.declare_dram_parameter("x", [C, N], F16, isOutput=False)
    wqT_d = nc.declare_dram_parameter("wqT", [C, HPC * D], F16, isOutput=False)
    wkT_d = nc.declare_dram_parameter("wkT", [C, HPC * D], F16, isOutput=False)
    wvT_d = nc.declare_dram_parameter("wvT", [C, HPC * D], F16, isOutput=False)
    woT_d = nc.declare_dram_parameter("woT", [HPC * D, C], F16, isOutput=False)
    qsks8_d = nc.declare_dram_parameter("qsks8", [128, 1], F32, isOutput=False)
    onesw_d = nc.declare_dram_parameter("onesw", [128, 33], F32R, isOutput=False)
    ones64r_d = nc.declare_dram_parameter("ones64r", [1, 64], F32R, isOutput=False)
    biasq_d = nc.declare_dram_parameter("biasq", [C, 1], F32, isOutput=False)
    out_d = nc.declare_dram_parameter("out", [C, N], F32, isOutput=True)

    NQT = N // 512  # 4 query chunks of 512
    NJ = N // 128  # 16 key chunks of 128
    NCT = C // 128  # 4 channel chunks of 128

    with tile.TileContext(nc) as tc:
        with (
            tc.tile_pool(name="const", bufs=1) as const,
            tc.tile_pool(name="persist", bufs=1) as persist,
            tc.tile_pool(name="dramp", bufs=1, space="DRAM") as dramp,
            # phase-2 SBUF pools live at the outer scope so their bytes are
            # disjoint from the phase-1 pools -- otherwise the first `at`
            # tile inherits WAR hazards on phase-1 tiles still being read
            tc.tile_pool(name="at", bufs=4) as atp,
            tc.tile_pool(name="nrm", bufs=4) as nrm,
            tc.tile_pool(name="fin", bufs=4) as finp,
        ):
            qsks8_sb = const.tile([128, 1], F32, name="qsks8", tag="qsks8")
            nc.sync.dma_start(qsks8_sb[:], qsks8_d[:])
            biasq_sb = const.tile([128, NCT], F32, name="biasq", tag="biasq")
            nc.sync.dma_start(
                biasq_sb[:], biasq_d[:].rearrange("(c p) o -> p (c o)", p=128)
            )
            # indicator weights: col 0 sums partitions 0-63 (head A), col 32
            # sums partitions 64-127 (head B); middle cols write zeros so the
            # [33, 512] sumsq psum rows land 32-aligned.
            ones_w = const.tile([128, 33], F32R, name="ones_w", tag="ones_w")
            nc.sync.dma_start(ones_w[:], onesw_d[:])
            ones64r = const.tile([1, 64], F32R, name="ones64r", tag="ones64r")
            nc.sync.dma_start(ones64r[:], ones64r_d[:])
            wo_sb = [
                const.tile([128, C], F16, name=f"wo{m}", tag=f"wo{m}")
                for m in range(2)
            ]

            # persistent tensors
            qn = [persist.tile([128, N], F16, name=f"qn{m}", tag=f"qn{m}") for m in range(2)]
            kn = [persist.tile([128, N], F16, name=f"kn{m}", tag=f"kn{m}") for m in range(2)]
            y = [
                [
                    persist.tile([128, 512], F16, name=f"y{m}_{qt}", tag=f"y{m}_{qt}")
                    for qt in range(4)
                ]
                for m in range(2)
            ]
            vext = persist.tile([128, NJ, HPC, D + 1], F16, name="vext", tag="vext")
            inv_dram = dramp.tile([8, N], F32, name="inv_dram", tag="inv_dram")
            # softmax-denominator ones column: DVE memset (gpsimd Q7 ops pay
            # ~6us of launch turnaround each -- keep that engine cold)
            nc.vector.memset(vext[:, :, :, D : D + 1], 1.0)

            # ---------------- phase 1: projections + norms ----------------
            with (
                tc.tile_pool(name="xw", bufs=1) as xw,
                tc.tile_pool(name="raw", bufs=1) as rawp,
                tc.tile_pool(name="sq", bufs=3) as sqp,
                tc.tile_pool(name="bb", bufs=4) as bbp,
                tc.tile_pool(name="prps", bufs=3, space="PSUM") as prps,
                tc.tile_pool(name="ssps", bufs=2, space="PSUM") as ssps,
            ):
                srt_tm = [
                    [
                        rawp.tile([33, N], F32, name=f"srt{t}{m}", tag=f"srt{t}{m}")
                        for m in range(2)
                    ]
                    for t in range(2)
                ]
                inv_tm = [
                    [
                        rawp.tile([33, N], F32, name=f"inv{t}{m}", tag=f"inv{t}{m}")
                        for m in range(2)
                    ]
                    for t in range(2)
                ]
                # DMA order tuned for earliest first matmul: wq, x[0], wk,
                # x[1..3], wv, wo; x rides [128,1024] half-row tiles so the
                # c-major projection chains can use f=1024 matmuls.
                wq_all = xw.tile([128, NCT, HPC * D], F16, name="wq_all", tag="wq_all")
                nc.scalar.dma_start(
                    wq_all[:], wqT_d[:].rearrange("(c p) d -> p c d", p=128)
                )
                wq_sb = [wq_all[:, c, :] for c in range(NCT)]
                dma_engs = [nc.sync, nc.scalar, nc.gpsimd, nc.sync]
                x2_sb = [[None, None] for _ in range(NCT)]
                for c in range(NCT):
                    for h in range(2):
                        t = xw.tile([128, 1024], F16, name=f"x{c}_{h}", tag=f"x{c}_{h}")
                        dma_engs[(2 * c + h) % 4].dma_start(
                            t[:],
                            x_d[c * 128 : (c + 1) * 128, h * 1024 : (h + 1) * 1024],
                        )
                        x2_sb[c][h] = t
                    if c == 0:
                        wk_all = xw.tile(
                            [128, NCT, HPC * D], F16, name="wk_all", tag="wk_all"
                        )
                        nc.scalar.dma_start(
                            wk_all[:], wkT_d[:].rearrange("(c p) d -> p c d", p=128)
                        )
                wk_sb = [wk_all[:, c, :] for c in range(NCT)]
                # v/out-proj weights ride after all of x: they are not
                # needed until the v projection / attention epilogue
                wv_all = xw.tile(
                    [128, NCT, HPC * D], F16, name="wv_all", tag="wv_all"
                )
                nc.scalar.dma_start(
                    wv_all[:], wvT_d[:].rearrange("(c p) d -> p c d", p=128)
                )
                wv_sb = [wv_all[:, c, :] for c in range(NCT)]
                for m in range(2):
                    nc.sync.dma_start(
                        wo_sb[m][:], woT_d[m * 128 : (m + 1) * 128, :]
                    )

                # c-major projection: one [128,1024] psum tile per half-row;
                # stationary weights reused across the f=1024 matmuls, so
                # LDWEIGHTS count drops 4x vs nt-major [128,512] chunks.
                # Evac/square emitted right after each group's stop; sumsq
                # matmul + sqrt one group late so the PE never waits on ACT.
                pend_ss = []
                evac_eng = [0]

                def emit_ss(limit):
                    while len(pend_ss) > limit:
                        sq_t, ti_, m_, h_ = pend_ss.pop(0)
                        for q in range(2):
                            ss = ssps.tile([33, 512], F32, name="ss", tag="ss")
                            nc.tensor.matmul(
                                ss[:],
                                lhsT=(ones_w[:]),
                                rhs=(sq_t[:, q * 512 : (q + 1) * 512]),
                                start=True,
                                stop=True,
                            )
                            nc.scalar.activation(
                                srt_tm[ti_][m_][
                                    :, (2 * h_ + q) * 512 : (2 * h_ + q + 1) * 512
                                ],
                                ss[:],
                                AF.Sqrt,
                            )

                def proj_group(m, w_sb, raws, ti):
                    prs = []
                    for h in range(2):
                        prs.append(prps.tile([128, 1024], F32, name="pr", tag="pr"))
                    for c in range(NCT):
                        # alternate the two psum tiles (different banks) so
                        # consecutive matmuls pipeline like the sim pairs
                        for q in range(2):
                            for h in range(2):
                                cs = slice(q * 512, (q + 1) * 512)
                                nc.tensor.matmul(
                                    prs[h][:, cs],
                                    lhsT=(w_sb[c][:, m * 128 : (m + 1) * 128]),
                                    rhs=(x2_sb[c][h][:, cs]),
                                    start=(c == 0),
                                    stop=(c == NCT - 1),
                                )
                    # q evacuations fold the qsks8 per-partition scale in
                    # (free on both engines); k evacuations are plain copies
                    scaled = raws is qn
                    for h in range(2):
                        dst = raws[m][:, h * 1024 : (h + 1) * 1024]
                        if evac_eng[0] % 2 == 0:
                            if scaled:
                                nc.vector.tensor_scalar(
                                    dst, prs[h][:], qsks8_sb[:], None, op0=OP.mult
                                )
                            else:
                                nc.vector.tensor_copy(dst, prs[h][:])
                        else:
                            nc.scalar.activation(
                                dst, prs[h][:], AF.Copy,
                                scale=qsks8_sb[:] if scaled else 1.0,
                            )
                        evac_eng[0] += 1
                        sq = sqp.tile([128, 1024], F32R, name="sq", tag="sq")
                        nc.scalar.activation(sq[:], prs[h][:], AF.Square)
                        pend_ss.append((sq, ti, m, h))

                def norm_head(m):
                    # reciprocals + the inverse-norm row DMA roundtrip with
                    # the 64-partition broadcast.
                    bts = []
                    for ti in range(2):
                        nc.vector.reciprocal_approx_fast(
                            inv_tm[ti][m][:], srt_tm[ti][m][:]
                        )
                        nc.sync.dma_start(
                            inv_dram[4 * ti + 2 * m : 4 * ti + 2 * m + 2, :],
                            inv_tm[ti][m][0:33:32, :],
                        )
                    bt_engs = [nc.sync, nc.scalar]
                    for ti in range(2):
                        rowA = 4 * ti + 2 * m
                        bt = bbp.tile([128, N], F32, name="bt", tag="bt")
                        eng = bt_engs[ti]
                        eng.dma_start(
                            bt[0:64, :].unsqueeze(1),
                            inv_dram[rowA : rowA + 1, :].partition_broadcast(64),
                        )
                        eng.dma_start(
                            bt[64:128, :].unsqueeze(1),
                            inv_dram[rowA + 1 : rowA + 2, :].partition_broadcast(64),
                        )
                        bts.append(bt)
                    return bts

                def norm_tail(m, bts):
                    # qsks8 already folded in at evacuation time; plain
                    # inverse-norm multiplies.  m=1's kn goes to gpsimd (its
                    # only op all kernel): the ~6us Q7 launch is hidden, and
                    # it keeps 2.2us off the DVE queue right when the first
                    # attention block starts.
                    nc.vector.tensor_tensor(
                        qn[m][:], qn[m][:], bts[0][:], OP.mult
                    )
                    if m == 0:
                        nc.vector.tensor_tensor(
                            kn[m][:], kn[m][:], bts[1][:], OP.mult
                        )
                    else:
                        nc.gpsimd.tensor_tensor(
                            kn[m][:], kn[m][:], bts[1][:], OP.mult
                        )

                # groups: q0, k0 | q1, k1; each group's sumsq chain runs one
                # group late; norm chains threaded between groups so the
                # inverse-norm DMA roundtrips overlap projection matmuls.
                proj_group(0, wq_sb, qn, 0)
                proj_group(0, wk_sb, kn, 1)
                emit_ss(1)
                proj_group(1, wq_sb, qn, 0)
                emit_ss(1)
                bts0 = norm_head(0)
                proj_group(1, wk_sb, kn, 1)
                emit_ss(1)
                norm_tail(0, bts0)

                # v projection; vext copies alternate ACT/DVE so neither
                # engine is the phase-1 straggler
                for nm_ in range(NJ):
                    psv = prps.tile([128, HPC * D], F32, name="prv", tag="pr")
                    for c in range(NCT):
                        nc.tensor.matmul(
                            psv[:],
                            lhsT=(
                                x2_sb[c][nm_ // 8][
                                    :, (nm_ % 8) * 128 : (nm_ % 8) * 128 + 128
                                ]
                            ),
                            rhs=(wv_sb[c][:]),
                            start=(c == 0),
                            stop=(c == NCT - 1),
                        )
                    if nm_ % 2 == 0:
                        nc.scalar.activation(
                            vext[:, nm_, :, 0:D],
                            psv[:].rearrange("p (h d) -> p h d", h=HPC),
                            AF.Copy,
                        )
                    else:
                        nc.vector.tensor_copy(
                            vext[:, nm_, :, 0:D],
                            psv[:].rearrange("p (h d) -> p h d", h=HPC),
                        )
                    if nm_ == 1:
                        emit_ss(0)
                    if nm_ == 3:
                        bts1 = norm_head(1)
                norm_tail(1, bts1)

            # ---------------- phase 2: attention + fused out-proj ----------
            # PSUM banks: sim 3 + o 4 + (pp|br shared) 1 = 8.  No gpsimd op
            # anywhere in this phase (each Q7 software op costs ~6us of
            # turnaround, which serialized the whole block handoff).
            with (
                tc.tile_pool(name="simps", bufs=3, space="PSUM") as simps,
                tc.tile_pool(name="ops", bufs=4, space="PSUM") as ops,
                tc.tile_pool(name="ppps", bufs=1, space="PSUM") as ppps,
            ):
                def out_proj_halves(qt, ct, from_ops=False):
                    qs_ = slice(qt * 512, (qt + 1) * 512)
                    box = {}

                    def start_half():
                        # the final qt borrows the (by then idle) 4-deep o
                        # pool so its four column chunks pipeline instead of
                        # serialising through the single pp bank
                        pool, tag = (ops, "o") if from_ops else (ppps, "pp")
                        pp = pool.tile([128, 512], F32, name="pp", tag=tag)
                        box["pp"] = pp
                        nc.tensor.matmul(
                            pp[:],
                            lhsT=(wo_sb[0][:, ct * 128 : (ct + 1) * 128]),
                            rhs=(y[0][qt][:]),
                            start=True,
                            stop=False,
                        )

                    def finish_half():
                        pp = box["pp"]
                        nc.tensor.matmul(
                            pp[:],
                            lhsT=(wo_sb[1][:, ct * 128 : (ct + 1) * 128]),
                            rhs=(y[1][qt][:]),
                            start=False,
                            stop=True,
                        )
                        ot = finp.tile([128, 512], F32, name="ot", tag="ot")
                        # bias applied here: Identity allows a [P,1] bias AP
                        # (Copy does not), so no extra matmul or DVE op
                        nc.scalar.activation(
                            ot[:], pp[:], AF.Identity,
                            bias=biasq_sb[:, ct : ct + 1],
                        )
                        nc.sync.dma_start(
                            out_d[ct * 128 : (ct + 1) * 128, qs_], ot[:]
                        )

                    return start_half, finish_half

                def av_pair(at_t, j_, oA, oB, hA, hB):
                    nc.tensor.matmul(
                        oA[:],
                        lhsT=(vext[:, j_, hA, :]),
                        rhs=(at_t[:, 0:512]),
                        start=(j_ == 0),
                        stop=(j_ == NJ - 1),
                    )
                    nc.tensor.matmul(
                        oB[:],
                        lhsT=(vext[:, j_, hB, :]),
                        rhs=(at_t[:, 512:1024]),
                        start=(j_ == 0),
                        stop=(j_ == NJ - 1),
                    )

                def normalize_stage1(oA, oB):
                    # ACT evacuates the denominator rows straight off the
                    # PSUM accumulators into F32R (offset-64 PSUM reads are
                    # only correct on ACT standard ops)
                    rs = []
                    for o_ps in (oA, oB):
                        rsb = nrm.tile([1, 512], F32R, name="rsb", tag="rsb")
                        nc.scalar.activation(rsb[:], o_ps[D : D + 1, :], AF.Copy)
                        rs.append(rsb)
                    return rs

                def normalize_stage2(oA, oB, rs, m, qt):
                    # PE broadcasts the raw denominator row (ones64r f32r
                    # matmul, 1 bank shared with the out-proj pool), DVE
                    # recips the [64,512] and multiplies the accumulator
                    # into fp16 y -- one PSUM operand per DVE op.
                    for o_ps, rsb, base in ((oA, rs[0], 0), (oB, rs[1], 64)):
                        br = ppps.tile([64, 512], F32, name="br", tag="pp")
                        nc.tensor.matmul(
                            br[:], lhsT=(ones64r[:]), rhs=(rsb[:]),
                            start=True, stop=True,
                        )
                        bri = nrm.tile([64, 512], F32, name="bri", tag="bri")
                        nc.vector.reciprocal_approx_fast(bri[:], br[:])
                        nc.vector.tensor_tensor(
                            y[m][qt][base : base + 64, :],
                            o_ps[0:D, :],
                            bri[:],
                            OP.mult,
                        )

                pend_av = [None]
                norm_pend = []
                pend_pp = []
                astep = [0]
                block_o = {}

                def flush_pend(last=False):
                    # stage2 of the previous block's normalize goes first so
                    # its br allocation precedes this block's pp allocations
                    while norm_pend and (last or norm_pend[0][0] <= astep[0]):
                        norm_pend.pop(0)[1]()
                    if pend_av[0] is None:
                        return
                    at_t, j_, m_, qt_ = pend_av[0]
                    pend_av[0] = None
                    if j_ == 0:
                        block_o[(m_, qt_)] = (
                            ops.tile([D + 1, 512], F32, name="oA", tag="o"),
                            ops.tile([D + 1, 512], F32, name="oB", tag="o"),
                        )
                    oA, oB = block_o[(m_, qt_)]
                    av_pair(at_t, j_, oA, oB, 2 * m_, 2 * m_ + 1)
                    if j_ == NJ - 1:
                        rs = normalize_stage1(oA, oB)
                        norm_pend.append(
                            (
                                astep[0] + 1,
                                lambda oA=oA, oB=oB, rs=rs, m_=m_, qt_=qt_: (
                                    normalize_stage2(oA, oB, rs, m_, qt_)
                                ),
                            )
                        )
                        if m_ == 1:
                            for ct in range(NCT):
                                for fn in out_proj_halves(qt_, ct, from_ops=last):
                                    if last:
                                        pend_pp.append((0, fn))
                                    else:
                                        pend_pp.append((astep[0] + 5, fn))

                for qt in range(NQT):
                    for m in range(2):
                        qs_ = slice(qt * 512, (qt + 1) * 512)
                        for j in range(NJ):
                            js = slice(j * 128, (j + 1) * 128)
                            simA = simps.tile([128, 512], F32, name="simA", tag="sim")
                            simB = simps.tile([128, 512], F32, name="simB", tag="sim")
                            nc.tensor.matmul(
                                simA[:],
                                lhsT=(kn[m][0:64, js]),
                                rhs=(qn[m][0:64, qs_]),
                                start=True,
                                stop=True,
                            )
                            nc.tensor.matmul(
                                simB[:],
                                lhsT=(kn[m][64:128, js]),
                                rhs=(qn[m][64:128, qs_]),
                                start=True,
                                stop=True,
                            )
                            flush_pend()
                            if pend_pp and astep[0] >= pend_pp[0][0]:
                                pend_pp.pop(0)[1]()
                            at = atp.tile([128, 1024], F16, name="at", tag="at")
                            # exact exp on ACT for head A
                            nc.scalar.activation(at[:, 0:512], simA[:], AF.Exp)
                            # Schraudolph fast-exp on DVE for head B
                            nc.vector.tensor_scalar(
                                at[:, 512:1024].bitcast(I16),
                                simB[:],
                                S_SCH,
                                B_SCH,
                                op0=OP.mult,
                                op1=OP.add,
                            )
                            pend_av[0] = (at, j, m, qt)
                            astep[0] += 1
                flush_pend(last=True)
                while norm_pend:
                    norm_pend.pop(0)[1]()
                while pend_pp:
                    pend_pp.pop(0)[1]()

    nc.finalize()
    return nc


def kernel(x, Wqkv, q_scale, k_scale, Wout, bout):
    global _CACHED_NC, LAST_RESULTS
    x = np.asarray(x, dtype=np.float32)
    Wqkv = np.asarray(Wqkv, dtype=np.float32)
    q_scale = np.asarray(q_scale, dtype=np.float32)
    k_scale = np.asarray(k_scale, dtype=np.float32)
    Wout = np.asarray(Wout, dtype=np.float32)
    bout = np.asarray(bout, dtype=np.float32)

    if _CACHED_NC is None:
        _CACHED_NC = build_nc()
    nc = _CACHED_NC

    H_DIM = HEADS * D
    qsks8 = np.tile((SCALE * q_scale * k_scale).astype(np.float32), 2)[:, None]
    qsks8 = np.ascontiguousarray(qsks8)
    biasq = np.ascontiguousarray((bout / 4.0).astype(np.float32)[:, None])
    onesw = np.zeros((128, 33), dtype=np.float32)
    onesw[0:64, 0] = 1.0
    onesw[64:128, 32] = 1.0

    in_maps = []
    for core in range(NCORES):
        b = core // 4
        h0 = HPC * (core % 4)
        rs = slice(h0 * D, h0 * D + HPC * D)
        wq = Wqkv[0:H_DIM][rs]
        wk = Wqkv[H_DIM : 2 * H_DIM][rs]
        wv = Wqkv[2 * H_DIM : 3 * H_DIM][rs]
        in_maps.append(
            {
                "x": np.ascontiguousarray(x[b]).astype(np.float16),
                "wqT": np.ascontiguousarray(wq.T).astype(np.float16),
                "wkT": np.ascontiguousarray(wk.T).astype(np.float16),
                "wvT": np.ascontiguousarray(wv.T).astype(np.float16),
                "woT": np.ascontiguousarray(Wout[:, rs].T).astype(np.float16),
                "qsks8": qsks8,
                "onesw": onesw,
                "ones64r": np.ones((1, 64), dtype=np.float32),
                "biasq": biasq,
            }
        )

    res = run_bass_kernel_spmd(
        nc,
        in_maps,
        core_ids=list(range(NCORES)),
        trace=bool(os.environ.get("BASS_TRACE")),
        **EXTRA_RUN_KWARGS,
    )
    LAST_RESULTS = res

    outs = [np.asarray(res.results[i]["out"], dtype=np.float32) for i in range(NCORES)]
    full = np.empty((B, C, N), dtype=np.float32)
    full[0] = outs[0] + outs[1] + outs[2] + outs[3]
    full[1] = outs[4] + outs[5] + outs[6] + outs[7]
    return full


# revision 38
# speedup vs baseline: 1.6829x; 1.0063x over previous
"""Trainium2 Bass kernel for cosine-similarity ("sparse") attention.

Reference computation (B=2, C=512, N=2048, H=16, D=64, SCALE=8):
    qkv = Wqkv @ x                          # 1x1 conv
    q,k,v -> [B,H,D,N]
    q = l2norm(q, over D) * q_scale ; k = l2norm(k, over D) * k_scale
    sim = (q^T k) * 8 ; attn = softmax(sim, over keys)
    out = Wout @ (attn @ v) + bout

Sharding: 32 (batch, head) pairs across 8 cores -> each core owns one batch
(b = core//4) and 4 heads (h0 = 4*(core%4)).  Each core projects q/k/v for
its heads, runs attention, and computes a partial output projection
Wout[:, its-heads] @ y + bout/4.  Host sums the 4 partials per batch.

Device-side schedule (per core):
  - fp16 for every 16-bit tensor (x, W, qn/kn, vext, at, y, wo): fp16's 10
    mantissa bits beat bf16 ~8x on this problem, which buys the error
    budget for the fast-exp trick below.
  - The [128,1024] softmax exp (2 heads x 512 queries per j-step) is
    split per head: ACT does head A with the exact Exp table; DVE does
    head B with a Schraudolph bit-trick exp: i16 = round(sim*1024*log2e
    + (15*1024 - 44)) via one tensor_scalar (fp32 PSUM -> int16), the
    int16 tile being the fp16 `at` tile bitcast.  ~+-4% on those columns,
    but softmax self-normalisation cancels the common mode; end-to-end
    rel err ~8e-3 (gate 2e-2).
  - sim is two per-head [128,512] PSUM tiles (bufs=4) so the engines
    never share a tile and block handoffs don't stall on bank rotation.
  - Softmax denominator: ones column in vext -> row D of the av
    accumulator.  Chain: ACT Copy evacuates rows [0:64] and row 64 (ACT
    handles partition-offset PSUM reads; custom-DVE ops and gpsimd
    partition_broadcast do NOT -- verified on HW), DVE recip, gpsimd
    broadcast, gpsimd multiply into fp16 y (gpsimd has no PSUM port).
  - out-proj bias rides the PSUM accumulation as a third matmul
    (bias_row x ones_row), so the evacuation is a plain ACT Copy (Copy
    shares the exp table set -- no ACT table thrash).
  - The av pair runs one j-step behind the sim pair ACROSS block
    boundaries; out-proj column chunks are staggered one per 2 j-steps;
    the last block flushes immediately to shorten the tail.
  - Softmax max-subtraction is skipped: sim = 8*cosine is in [-8, 8]
    (which also keeps the Schraudolph i16 strictly positive).
  - l2norm: ones-indicator matmul for sumsq, fused PSUM->SBUF Sqrt on
    ACT, DVE reciprocal, inverse-norm broadcast via a DMA roundtrip.
  - Startup: x rides one DMA per 128-partition chunk; wq first so the
    first projection starts as early as possible.
"""

import os
import sys

import numpy as np

sys.path.insert(0, "/opt/trn_rl_repo")

import concourse.bass as bass  # noqa: E402
import concourse.mybir as mybir  # noqa: E402
from concourse import bacc, tile  # noqa: E402
from concourse.bass_utils import run_bass_kernel_spmd  # noqa: E402

F32 = mybir.dt.float32
F32R = mybir.dt.float32r
F16 = mybir.dt.float16
I16 = mybir.dt.int16
AF = mybir.ActivationFunctionType
OP = mybir.AluOpType

B, C, N = 2, 512, 2048
HEADS, D = 16, 64
SCALE = 8.0
NCORES = 8
HPC = 4  # heads per core

# Schraudolph fast-exp constants (fp16 bit layout):
# i16 = round(sim * S_SCH + B_SCH); bits reinterpreted as fp16 ~= e^sim.
S_SCH = 1024.0 / float(np.log(2.0))
B_SCH = 15.0 * 1024.0 - 44.0

_CACHED_NC = None
LAST_RESULTS = None
EXTRA_RUN_KWARGS = {}


def build_nc():
    nc = bacc.Bacc(None, target_bir_lowering=False)

    x_d = nc.declare_dram_parameter("x", [C, N], F16, isOutput=False)
    wqT_d = nc.declare_dram_parameter("wqT", [C, HPC * D], F16, isOutput=False)
    wkT_d = nc.declare_dram_parameter("wkT", [C, HPC * D], F16, isOutput=False)
    wvT_d = nc.declare_dram_parameter("wvT", [C, HPC * D], F16, isOutput=False)
    woT_d = nc.declare_dram_parameter("woT", [HPC * D, C], F16, isOutput=False)
    qsks8_d = nc.declare_dram_parameter("qsks8", [128, 1], F32, isOutput=False)
    onesw_d = nc.declare_dram_parameter("onesw", [128, 33], F32R, isOutput=False)
    ones64r_d = nc.declare_dram_parameter("ones64r", [1, 64], F32R, isOutput=False)
    biasq_d = nc.declare_dram_parameter("biasq", [C, 1], F32, isOutput=False)
    out_d = nc.declare_dram_parameter("out", [C, N], F32, isOutput=True)

    NQT = N // 512  # 4 query chunks of 512
    NJ = N // 128  # 16 key chunks of 128
    NCT = C // 128  # 4 channel chunks of 128

    with tile.TileContext(nc) as tc:
        with (
            tc.tile_pool(name="const", bufs=1) as const,
            tc.tile_pool(name="persist", bufs=1) as persist,
            tc.tile_pool(name="dramp", bufs=1, space="DRAM") as dramp,
            # phase-2 SBUF pools live at the outer scope so their bytes are
            # disjoint from the phase-1 pools -- otherwise the first `at`
            # tile inherits WAR hazards on phase-1 tiles still being read
            tc.tile_pool(name="at", bufs=4) as atp,
            tc.tile_pool(name="nrm", bufs=4) as nrm,
            tc.tile_pool(name="fin", bufs=4) as finp,
            # bb holds the broadcast inverse-norm tiles; outer scope so the
            # phase-1 pool close (and phase-2 PSUM pool open) never waits on
            # the m=1 norm-tail multiplies that run into early attention
            tc.tile_pool(name="bb", bufs=4) as bbp,
        ):
            qsks8_sb = const.tile([128, 1], F32, name="qsks8", tag="qsks8")
            nc.sync.dma_start(qsks8_sb[:], qsks8_d[:])
            biasq_sb = const.tile([128, NCT], F32, name="biasq", tag="biasq")
            nc.sync.dma_start(
                biasq_sb[:], biasq_d[:].rearrange("(c p) o -> p (c o)", p=128)
            )
            # indicator weights: col 0 sums partitions 0-63 (head A), col 32
            # sums partitions 64-127 (head B); middle cols write zeros so the
            # [33, 512] sumsq psum rows land 32-aligned.
            ones_w = const.tile([128, 33], F32R, name="ones_w", tag="ones_w")
            nc.sync.dma_start(ones_w[:], onesw_d[:])
            ones64r = const.tile([1, 64], F32R, name="ones64r", tag="ones64r")
            nc.sync.dma_start(ones64r[:], ones64r_d[:])
            wo_sb = [
                const.tile([128, C], F16, name=f"wo{m}", tag=f"wo{m}")
                for m in range(2)
            ]

            # persistent tensors
            qn = [persist.tile([128, N], F16, name=f"qn{m}", tag=f"qn{m}") for m in range(2)]
            kn = [persist.tile([128, N], F16, name=f"kn{m}", tag=f"kn{m}") for m in range(2)]
            y = [
                [
                    persist.tile([128, 512], F16, name=f"y{m}_{qt}", tag=f"y{m}_{qt}")
                    for qt in range(4)
                ]
                for m in range(2)
            ]
            vext = persist.tile([128, NJ, HPC, D + 1], F16, name="vext", tag="vext")
            inv_dram = dramp.tile([8, N], F32, name="inv_dram", tag="inv_dram")
            # softmax-denominator ones column: DVE memset (gpsimd Q7 ops pay
            # ~6us of launch turnaround each -- keep that engine cold)
            nc.vector.memset(vext[:, :, :, D : D + 1], 1.0)

            # ---------------- phase 1: projections + norms ----------------
            with (
                tc.tile_pool(name="xw", bufs=1) as xw,
                tc.tile_pool(name="raw", bufs=1) as rawp,
                tc.tile_pool(name="sq", bufs=3) as sqp,
                tc.tile_pool(name="prps", bufs=3, space="PSUM") as prps,
                tc.tile_pool(name="ssps", bufs=2, space="PSUM") as ssps,
            ):
                srt_tm = [
                    [
                        rawp.tile([33, N], F32, name=f"srt{t}{m}", tag=f"srt{t}{m}")
                        for m in range(2)
                    ]
                    for t in range(2)
                ]
                inv_tm = [
                    [
                        rawp.tile([33, N], F32, name=f"inv{t}{m}", tag=f"inv{t}{m}")
                        for m in range(2)
                    ]
                    for t in range(2)
                ]
                # DMA order tuned for earliest first matmul: wq, x[0], wk,
                # x[1..3], wv, wo; x rides [128,1024] half-row tiles so the
                # c-major projection chains can use f=1024 matmuls.
                wq_all = xw.tile([128, NCT, HPC * D], F16, name="wq_all", tag="wq_all")
                nc.scalar.dma_start(
                    wq_all[:], wqT_d[:].rearrange("(c p) d -> p c d", p=128)
                )
                wq_sb = [wq_all[:, c, :] for c in range(NCT)]
                dma_engs = [nc.sync, nc.scalar, nc.gpsimd, nc.sync]
                x2_sb = [[None, None] for _ in range(NCT)]
                for c in range(NCT):
                    for h in range(2):
                        t = xw.tile([128, 1024], F16, name=f"x{c}_{h}", tag=f"x{c}_{h}")
                        dma_engs[(2 * c + h) % 4].dma_start(
                            t[:],
                            x_d[c * 128 : (c + 1) * 128, h * 1024 : (h + 1) * 1024],
                        )
                        x2_sb[c][h] = t
                    if c == 0:
                        wk_all = xw.tile(
                            [128, NCT, HPC * D], F16, name="wk_all", tag="wk_all"
                        )
                        nc.scalar.dma_start(
                            wk_all[:], wkT_d[:].rearrange("(c p) d -> p c d", p=128)
                        )
                wk_sb = [wk_all[:, c, :] for c in range(NCT)]
                # v/out-proj weights ride after all of x: they are not
                # needed until the v projection / attention epilogue
                wv_all = xw.tile(
                    [128, NCT, HPC * D], F16, name="wv_all", tag="wv_all"
                )
                nc.scalar.dma_start(
                    wv_all[:], wvT_d[:].rearrange("(c p) d -> p c d", p=128)
                )
                wv_sb = [wv_all[:, c, :] for c in range(NCT)]
                for m in range(2):
                    nc.sync.dma_start(
                        wo_sb[m][:], woT_d[m * 128 : (m + 1) * 128, :]
                    )

                # c-major projection: one [128,1024] psum tile per half-row;
                # stationary weights reused across the f=1024 matmuls, so
                # LDWEIGHTS count drops 4x vs nt-major [128,512] chunks.
                # Evac/square emitted right after each group's stop; sumsq
                # matmul + sqrt one group late so the PE never waits on ACT.
                pend_ss = []
                evac_eng = [0]

                def emit_ss(limit):
                    while len(pend_ss) > limit:
                        sq_t, ti_, m_, h_ = pend_ss.pop(0)
                        for q in range(2):
                            ss = ssps.tile([33, 512], F32, name="ss", tag="ss")
                            nc.tensor.matmul(
                                ss[:],
                                lhsT=(ones_w[:]),
                                rhs=(sq_t[:, q * 512 : (q + 1) * 512]),
                                start=True,
                                stop=True,
                            )
                            nc.scalar.activation(
                                srt_tm[ti_][m_][
                                    :, (2 * h_ + q) * 512 : (2 * h_ + q + 1) * 512
                                ],
                                ss[:],
                                AF.Sqrt,
                            )

                def group_mm(pr, w_sb, m, c, h, q, stop):
                    cs = slice(q * 512, (q + 1) * 512)
                    nc.tensor.matmul(
                        pr[:, cs],
                        lhsT=(w_sb[c][:, m * 128 : (m + 1) * 128]),
                        rhs=(x2_sb[c][h][:, cs]),
                        start=(c == 0),
                        stop=stop,
                    )

                def evac_sq(pr, raws, m, ti, h):
                    # q evacuations fold the qsks8 per-partition scale in
                    # (free on both engines); k evacuations are plain copies
                    scaled = raws is qn
                    dst = raws[m][:, h * 1024 : (h + 1) * 1024]
                    if evac_eng[0] % 2 == 0:
                        if scaled:
                            nc.vector.tensor_scalar(
                                dst, pr[:], qsks8_sb[:], None, op0=OP.mult
                            )
                        else:
                            nc.vector.tensor_copy(dst, pr[:])
                    else:
                        nc.scalar.activation(
                            dst, pr[:], AF.Copy,
                            scale=qsks8_sb[:] if scaled else 1.0,
                        )
                    evac_eng[0] += 1
                    sq = sqp.tile([128, 1024], F32R, name="sq", tag="sq")
                    nc.scalar.activation(sq[:], pr[:], AF.Square)
                    pend_ss.append((sq, ti, m, h))

                def proj_group(m, w_sb, raws, ti):
                    prs = [
                        prps.tile([128, 1024], F32, name="pr", tag="pr")
                        for _ in range(2)
                    ]
                    for c in range(NCT):
                        # alternate the two psum tiles (different banks) so
                        # consecutive matmuls pipeline like the sim pairs
                        for q in range(2):
                            for h in range(2):
                                group_mm(prs[h], w_sb, m, c, h, q,
                                         stop=(c == NCT - 1))
                    for h in range(2):
                        evac_sq(prs[h], raws, m, ti, h)

                def norm_head(m):
                    # reciprocals + the inverse-norm row DMA roundtrip with
                    # the 64-partition broadcast.
                    bts = []
                    for ti in range(2):
                        nc.vector.reciprocal_approx_fast(
                            inv_tm[ti][m][:], srt_tm[ti][m][:]
                        )
                        nc.sync.dma_start(
                            inv_dram[4 * ti + 2 * m : 4 * ti + 2 * m + 2, :],
                            inv_tm[ti][m][0:33:32, :],
                        )
                    bt_engs = [nc.sync, nc.scalar]
                    for ti in range(2):
                        rowA = 4 * ti + 2 * m
                        bt = bbp.tile([128, N], F32, name="bt", tag="bt")
                        eng = bt_engs[ti]
                        eng.dma_start(
                            bt[0:64, :].unsqueeze(1),
                            inv_dram[rowA : rowA + 1, :].partition_broadcast(64),
                        )
                        eng.dma_start(
                            bt[64:128, :].unsqueeze(1),
                            inv_dram[rowA + 1 : rowA + 2, :].partition_broadcast(64),
                        )
                        bts.append(bt)
                    return bts

                def norm_tail(m, bts):
                    # qsks8 already folded in at evacuation time; plain
                    # inverse-norm multiplies.  m=1's kn goes to gpsimd (its
                    # only op all kernel): the ~6us Q7 launch is hidden, and
                    # it keeps 2.2us off the DVE queue right when the first
                    # attention block starts.
                    nc.vector.tensor_tensor(
                        qn[m][:], qn[m][:], bts[0][:], OP.mult
                    )
                    if m == 0:
                        nc.vector.tensor_tensor(
                            kn[m][:], kn[m][:], bts[1][:], OP.mult
                        )
                    else:
                        nc.gpsimd.tensor_tensor(
                            kn[m][:], kn[m][:], bts[1][:], OP.mult
                        )

                # m=0: q and the first k half interleave around the x DMA
                # tail (the c=3 chunk lands ~10us in; this keeps the PE fed
                # with c<3 work from both groups in the meantime).  3 psum
                # tiles live: q's two halves + k's first half.
                prq = [
                    prps.tile([128, 1024], F32, name="pr", tag="pr")
                    for _ in range(2)
                ]
                for c in range(NCT - 1):
                    for q_ in range(2):
                        for h in range(2):
                            group_mm(prq[h], wq_sb, 0, c, h, q_, stop=False)
                prkA = prps.tile([128, 1024], F32, name="pr", tag="pr")
                for c in range(NCT - 1):
                    for q_ in range(2):
                        group_mm(prkA, wk_sb, 0, c, 0, q_, stop=False)
                for q_ in range(2):
                    for h in range(2):
                        group_mm(prq[h], wq_sb, 0, NCT - 1, h, q_, stop=True)
                for h in range(2):
                    evac_sq(prq[h], qn, 0, 0, h)
                for q_ in range(2):
                    group_mm(prkA, wk_sb, 0, NCT - 1, 0, q_, stop=True)
                evac_sq(prkA, kn, 0, 1, 0)
                prkB = prps.tile([128, 1024], F32, name="pr", tag="pr")
                for c in range(NCT):
                    for q_ in range(2):
                        group_mm(prkB, wk_sb, 0, c, 1, q_, stop=(c == NCT - 1))
                evac_sq(prkB, kn, 0, 1, 1)
                emit_ss(1)
                proj_group(1, wq_sb, qn, 0)
                emit_ss(1)
                bts0 = norm_head(0)
                proj_group(1, wk_sb, kn, 1)
                emit_ss(1)
                norm_tail(0, bts0)

                # v projection; vext copies alternate ACT/DVE so neither
                # engine is the phase-1 straggler
                for nm_ in range(NJ):
                    psv = prps.tile([128, HPC * D], F32, name="prv", tag="pr")
                    for c in range(NCT):
                        nc.tensor.matmul(
                            psv[:],
                            lhsT=(
                                x2_sb[c][nm_ // 8][
                                    :, (nm_ % 8) * 128 : (nm_ % 8) * 128 + 128
                                ]
                            ),
                            rhs=(wv_sb[c][:]),
                            start=(c == 0),
                            stop=(c == NCT - 1),
                        )
                    if nm_ % 2 == 0:
                        nc.scalar.activation(
                            vext[:, nm_, :, 0:D],
                            psv[:].rearrange("p (h d) -> p h d", h=HPC),
                            AF.Copy,
                        )
                    else:
                        nc.vector.tensor_copy(
                            vext[:, nm_, :, 0:D],
                            psv[:].rearrange("p (h d) -> p h d", h=HPC),
                        )
                    if nm_ == 1:
                        emit_ss(0)
                    if nm_ == 3:
                        bts1 = norm_head(1)
                    if nm_ == 10:
                        # m=1 norm tail emitted while the v projection still
                        # runs: the DVE executes it as soon as the broadcast
                        # DMA lands, well before the m=1 attention blocks;
                        # nothing at the phase boundary waits on it
                        norm_tail(1, bts1)

            # ---------------- phase 2: attention + fused out-proj ----------
            # PSUM banks: sim 3 + o 4 + (pp|br shared) 1 = 8.  No gpsimd op
            # anywhere in this phase (each Q7 software op costs ~6us of
            # turnaround, which serialized the whole block handoff).
            with (
                tc.tile_pool(name="simps", bufs=3, space="PSUM") as simps,
                tc.tile_pool(name="ops", bufs=4, space="PSUM") as ops,
                tc.tile_pool(name="ppps", bufs=1, space="PSUM") as ppps,
            ):
                def out_proj_halves(qt, ct, from_ops=False):
                    qs_ = slice(qt * 512, (qt + 1) * 512)
                    box = {}

                    def start_half():
                        # the final qt borrows the (by then idle) 4-deep o
                        # pool so its four column chunks pipeline instead of
                        # serialising through the single pp bank
                        pool, tag = (ops, "o") if from_ops else (ppps, "pp")
                        pp = pool.tile([128, 512], F32, name="pp", tag=tag)
                        box["pp"] = pp
                        nc.tensor.matmul(
                            pp[:],
                            lhsT=(wo_sb[0][:, ct * 128 : (ct + 1) * 128]),
                            rhs=(y[0][qt][:]),
                            start=True,
                            stop=False,
                        )

                    def finish_half():
                        pp = box["pp"]
                        nc.tensor.matmul(
                            pp[:],
                            lhsT=(wo_sb[1][:, ct * 128 : (ct + 1) * 128]),
                            rhs=(y[1][qt][:]),
                            start=False,
                            stop=True,
                        )
                        ot = finp.tile([128, 512], F32, name="ot", tag="ot")
                        # bias applied at evacuation; engines alternate by
                        # column chunk so neither exp stream eats both copies
                        if ct % 2 == 0:
                            nc.scalar.activation(
                                ot[:], pp[:], AF.Identity,
                                bias=biasq_sb[:, ct : ct + 1],
                            )
                        else:
                            nc.vector.tensor_scalar_add(
                                ot[:], pp[:], biasq_sb[:, ct : ct + 1]
                            )
                        nc.sync.dma_start(
                            out_d[ct * 128 : (ct + 1) * 128, qs_], ot[:]
                        )

                    return start_half, finish_half

                def av_pair(at_t, j_, oA, oB, hA, hB):
                    nc.tensor.matmul(
                        oA[:],
                        lhsT=(vext[:, j_, hA, :]),
                        rhs=(at_t[:, 0:512]),
                        start=(j_ == 0),
                        stop=(j_ == NJ - 1),
                    )
                    nc.tensor.matmul(
                        oB[:],
                        lhsT=(vext[:, j_, hB, :]),
                        rhs=(at_t[:, 512:1024]),
                        start=(j_ == 0),
                        stop=(j_ == NJ - 1),
                    )

                def normalize_stage1(oA, oB):
                    # ACT evacuates the denominator rows straight off the
                    # PSUM accumulators into F32R (offset-64 PSUM reads are
                    # only correct on ACT standard ops)
                    rs = []
                    for o_ps in (oA, oB):
                        rsb = nrm.tile([1, 512], F32R, name="rsb", tag="rsb")
                        nc.scalar.activation(rsb[:], o_ps[D : D + 1, :], AF.Copy)
                        rs.append(rsb)
                    return rs

                def normalize_stage2(o_ps, rsb, m, qt, base):
                    # PE broadcasts the raw denominator row (ones64r f32r
                    # matmul, 1 bank shared with the out-proj pool), DVE
                    # recips the [64,512] and multiplies the accumulator
                    # into fp16 y -- one PSUM operand per DVE op.  One half
                    # per j-step so the DVE burst stays under the slack.
                    br = ppps.tile([64, 512], F32, name="br", tag="pp")
                    nc.tensor.matmul(
                        br[:], lhsT=(ones64r[:]), rhs=(rsb[:]),
                        start=True, stop=True,
                    )
                    bri = nrm.tile([64, 512], F32, name="bri", tag="bri")
                    nc.vector.reciprocal_approx_fast(bri[:], br[:])
                    nc.vector.tensor_tensor(
                        y[m][qt][base : base + 64, :],
                        o_ps[0:D, :],
                        bri[:],
                        OP.mult,
                    )

                pend_av = [None]
                norm_pend = []
                pend_pp = []
                astep = [0]
                block_o = {}

                def flush_pend(last=False):
                    # stage2 of the previous block's normalize goes first so
                    # its br allocation precedes this block's pp allocations
                    while norm_pend and (last or norm_pend[0][0] <= astep[0]):
                        norm_pend.pop(0)[1]()
                    if pend_av[0] is None:
                        return
                    at_t, j_, m_, qt_ = pend_av[0]
                    pend_av[0] = None
                    if j_ == 0:
                        block_o[(m_, qt_)] = (
                            ops.tile([D + 1, 512], F32, name="oA", tag="o"),
                            ops.tile([D + 1, 512], F32, name="oB", tag="o"),
                        )
                    oA, oB = block_o[(m_, qt_)]
                    av_pair(at_t, j_, oA, oB, 2 * m_, 2 * m_ + 1)
                    if j_ == NJ - 1:
                        rs = normalize_stage1(oA, oB)
                        for i, (o_ps, base) in enumerate(((oA, 0), (oB, 64))):
                            norm_pend.append(
                                (
                                    astep[0] + 1 + i,
                                    lambda o_ps=o_ps, rsb=rs[i], m_=m_, qt_=qt_,
                                    base=base: normalize_stage2(
                                        o_ps, rsb, m_, qt_, base
                                    ),
                                )
                            )
                        if m_ == 1:
                            for ct in range(NCT):
                                for fn in out_proj_halves(qt_, ct, from_ops=last):
                                    if last:
                                        pend_pp.append((0, fn))
                                    else:
                                        pend_pp.append((astep[0] + 5, fn))

                for qt in range(NQT):
                    for m in range(2):
                        qs_ = slice(qt * 512, (qt + 1) * 512)
                        for j in range(NJ):
                            js = slice(j * 128, (j + 1) * 128)
                            simA = simps.tile([128, 512], F32, name="simA", tag="sim")
                            simB = simps.tile([128, 512], F32, name="simB", tag="sim")
                            nc.tensor.matmul(
                                simA[:],
                                lhsT=(kn[m][0:64, js]),
                                rhs=(qn[m][0:64, qs_]),
                                start=True,
                                stop=True,
                            )
                            nc.tensor.matmul(
                                simB[:],
                                lhsT=(kn[m][64:128, js]),
                                rhs=(qn[m][64:128, qs_]),
                                start=True,
                                stop=True,
                            )
                            flush_pend()
                            if pend_pp and astep[0] >= pend_pp[0][0]:
                                pend_pp.pop(0)[1]()
                            at = atp.tile([128, 1024], F16, name="at", tag="at")
                            # exact exp on ACT for head A
                            nc.scalar.activation(at[:, 0:512], simA[:], AF.Exp)
                            # Schraudolph fast-exp on DVE for head B
                            nc.vector.tensor_scalar(
                                at[:, 512:1024].bitcast(I16),
                                simB[:],
                                S_SCH,
                                B_SCH,
                                op0=OP.mult,
                                op1=OP.add,
                            )
                            pend_av[0] = (at, j, m, qt)
                            astep[0] += 1
                flush_pend(last=True)
                while norm_pend:
                    norm_pend.pop(0)[1]()
                while pend_pp:
                    pend_pp.pop(0)[1]()

    nc.finalize()
    return nc


def kernel(x, Wqkv, q_scale, k_scale, Wout, bout):
    global _CACHED_NC, LAST_RESULTS
    x = np.asarray(x, dtype=np.float32)
    Wqkv = np.asarray(Wqkv, dtype=np.float32)
    q_scale = np.asarray(q_scale, dtype=np.float32)
    k_scale = np.asarray(k_scale, dtype=np.float32)
    Wout = np.asarray(Wout, dtype=np.float32)
    bout = np.asarray(bout, dtype=np.float32)

    if _CACHED_NC is None:
        _CACHED_NC = build_nc()
    nc = _CACHED_NC

    H_DIM = HEADS * D
    qsks8 = np.tile((SCALE * q_scale * k_scale).astype(np.float32), 2)[:, None]
    qsks8 = np.ascontiguousarray(qsks8)
    biasq = np.ascontiguousarray((bout / 4.0).astype(np.float32)[:, None])
    onesw = np.zeros((128, 33), dtype=np.float32)
    onesw[0:64, 0] = 1.0
    onesw[64:128, 32] = 1.0

    in_maps = []
    for core in range(NCORES):
        b = core // 4
        h0 = HPC * (core % 4)
        rs = slice(h0 * D, h0 * D + HPC * D)
        wq = Wqkv[0:H_DIM][rs]
        wk = Wqkv[H_DIM : 2 * H_DIM][rs]
        wv = Wqkv[2 * H_DIM : 3 * H_DIM][rs]
        in_maps.append(
            {
                "x": np.ascontiguousarray(x[b]).astype(np.float16),
                "wqT": np.ascontiguousarray(wq.T).astype(np.float16),
                "wkT": np.ascontiguousarray(wk.T).astype(np.float16),
                "wvT": np.ascontiguousarray(wv.T).astype(np.float16),
                "woT": np.ascontiguousarray(Wout[:, rs].T).astype(np.float16),
                "qsks8": qsks8,
                "onesw": onesw,
                "ones64r": np.ones((1, 64), dtype=np.float32),
                "biasq": biasq,
            }
        )

    res = run_bass_kernel_spmd(
        nc,
        in_maps,
        core_ids=list(range(NCORES)),
        trace=bool(os.environ.get("BASS_TRACE")),
        **EXTRA_RUN_KWARGS,
    )
    LAST_RESULTS = res

    outs = [np.asarray(res.results[i]["out"], dtype=np.float32) for i in range(NCORES)]
    full = np.empty((B, C, N), dtype=np.float32)
    full[0] = outs[0] + outs[1] + outs[2] + outs[3]
    full[1] = outs[4] + outs[5] + outs[6] + outs[7]
    return full
